# revision 1
# baseline (speedup 1.0000x reference)
"""Causal GQA self-attention (RoPE) Trainium2 Bass kernel, 8-core SPMD.

Sharding: core c -> (b = c//4, g = c%4).  Data-parallel over batch B=2,
tensor-parallel over the 4 KV groups (4 query heads + 1 KV head each).
Each core computes a partial output y_bg = attn_out_g @ Wo[:, g-block].T
for its batch; the host sums the 4 group partials per batch (row-parallel
linear unshard).

Per-core device kernel (all matmuls bf16, f32 PSUM accumulation):
  xT  <- DMA-transpose of x (bf16)                 [d, l]
  qT/kT = W @ xT projections (+RoPE via signed-perm matmul + cos/sin mults)
  v   = (Wv @ xT).T via PE transpose               [l, hd]
  per (head, 512-query chunk): S^T = K^T q chunks per 128-key tile,
    exp on ACT (scale=1/sqrt(hd) folded), causal handled by column-slicing
    + one triangular mask multiply on the diagonal tile,
    sum over keys + attn@V via PE matmuls (ones / v stationary),
    normalize with DVE reciprocal (sums replicated across partitions).
  y_partial = oT @ WoT accumulated over heads, DMA out as f32.
"""

import math
import sys

import numpy as np

try:
    import concourse.bass as bass  # noqa: F401
except ImportError:  # pragma: no cover
    sys.path.insert(0, "/opt/trn_rl_repo")
    import concourse.bass as bass  # noqa: F401

import ml_dtypes

import concourse.bacc as bacc
import concourse.mybir as mybir
import concourse.tile as tile
from concourse.bass_utils import run_bass_kernel_spmd

BF16 = ml_dtypes.bfloat16
F32 = np.float32

B, L, D = 2, 2048, 2048
HD = 128          # head dim
NHL = 4           # query heads per core (one KV group)
P = 128
NDT = D // P      # 16 d-tiles
NKT = L // P      # 16 key tiles
NLC = L // 512    # 4 512-wide l chunks
SM_SCALE = 1.0 / math.sqrt(HD)

_BF = mybir.dt.bfloat16
_F32 = mybir.dt.float32

# 1 = xT+projections+RoPE only, 2 = +attention, 3 = full (perf probing knob)
PHASE_LIMIT = 3


def build_nc():
    nc = bacc.Bacc("TRN2", target_bir_lowering=False, debug=False,
                   enable_asserts=False)

    x_d = nc.dram_tensor("x", [L, D], _BF, kind="ExternalInput").ap()
    wq_d = nc.dram_tensor("wq", [P, NDT, 512], _BF, kind="ExternalInput").ap()
    wk_d = nc.dram_tensor("wk", [P, NDT, 128], _BF, kind="ExternalInput").ap()
    wv_d = nc.dram_tensor("wv", [P, NDT, 128], _BF, kind="ExternalInput").ap()
    wo_d = nc.dram_tensor("wo", [P, NHL, L], _BF, kind="ExternalInput").ap()
    cos_d = nc.dram_tensor("cosT", [P, L], _BF, kind="ExternalInput").ap()
    sin_d = nc.dram_tensor("sinT", [P, L], _BF, kind="ExternalInput").ap()
    perm_d = nc.dram_tensor("perm", [P, P], _BF, kind="ExternalInput").ap()
    ones_d = nc.dram_tensor("ones", [P, P], _BF, kind="ExternalInput").ap()
    tri_d = nc.dram_tensor("tri", [P, P], _BF, kind="ExternalInput").ap()
    id_d = nc.dram_tensor("ident", [P, P], _BF, kind="ExternalInput").ap()
    y_d = nc.dram_tensor("y", [L, D], _F32, kind="ExternalOutput").ap()

    with tile.TileContext(nc) as tc:
        _body(nc, tc, x_d, wq_d, wk_d, wv_d, wo_d, cos_d, sin_d,
              perm_d, ones_d, tri_d, id_d, y_d)
    nc.compile()
    return nc


def _body(nc, tc, x_d, wq_d, wk_d, wv_d, wo_d, cos_d, sin_d,
          perm_d, ones_d, tri_d, id_d, y_d):
    from contextlib import ExitStack
    ctx = ExitStack()
    with ctx:
        pp = ctx.enter_context(tc.tile_pool(name="persist", bufs=1))
        wsb = ctx.enter_context(tc.tile_pool(name="wsb", bufs=2))

        xT = pp.tile([P, NDT, L], _BF, tag="xT")
        wq_sb = pp.tile([P, NDT, 512], _BF, tag="wq")
        wk_sb = pp.tile([P, NDT, 128], _BF, tag="wk")
        wv_sb = pp.tile([P, NDT, 128], _BF, tag="wv")
        wo_sb = pp.tile([P, NHL, L], _BF, tag="wo")
        cos_sb = pp.tile([P, L], _BF, tag="cos")
        sin_sb = pp.tile([P, L], _BF, tag="sin")
        perm_sb = pp.tile([P, P], _BF, tag="perm")
        ones_sb = pp.tile([P, P], _BF, tag="ones")
        tri_sb = pp.tile([P, P], _BF, tag="tri")
        id_sb = pp.tile([P, P], _BF, tag="ident")
        qT = pp.tile([P, NHL, L], _BF, tag="qT")
        kT = pp.tile([P, L], _BF, tag="kT")
        vn = pp.tile([P, NKT, 128], _BF, tag="vn")
        oT = pp.tile([P, NHL, L], _BF, tag="oT")

        nc.sync.dma_start(wq_sb[:], wq_d[:])
        nc.sync.dma_start(wk_sb[:], wk_d[:])
        nc.sync.dma_start(wv_sb[:], wv_d[:])
        nc.sync.dma_start(wo_sb[:], wo_d[:])
        nc.sync.dma_start(cos_sb[:], cos_d[:])
        nc.sync.dma_start(sin_sb[:], sin_d[:])
        nc.sync.dma_start(perm_sb[:], perm_d[:])
        nc.sync.dma_start(ones_sb[:], ones_d[:])
        nc.sync.dma_start(tri_sb[:], tri_d[:])
        nc.sync.dma_start(id_sb[:], id_d[:])

        # ---- projections + RoPE.  e-tiles: 0..3 = q heads, 4 = k, 5 = v
        with tc.tile_pool(name="ppr", bufs=1, space="PSUM") as ppr:
            # x -> xT via PE transpose, 4 dti blocks per PSUM bank
            for lt in range(NKT):
                xrow = wsb.tile([P, D], _BF, tag="xrow", bufs=3,
                                name=f"xrow_{lt}")
                nc.sync.dma_start(xrow[:], x_d[lt * P:(lt + 1) * P, :])
                for j in range(4):
                    xp = ppr.tile([P, 512], _BF, tag="vtp", bufs=2,
                                  name=f"xp_{lt}_{j}")
                    for i in range(4):
                        dti = 4 * j + i
                        nc.tensor.matmul(
                            xp[:, i * P:(i + 1) * P],
                            xrow[:, dti * P:(dti + 1) * P], id_sb[:],
                            is_transpose=True, skip_group_check=True)
                    nc.vector.tensor_copy(
                        xT[:, 4 * j:4 * j + 4, lt * P:(lt + 1) * P],
                        xp[:].rearrange("p (a b) -> p a b", a=4))
            for et in range(6):
                if et < 4:
                    w_sl = lambda d_: wq_sb[:, d_, et * 128:(et + 1) * 128]
                elif et == 4:
                    w_sl = lambda d_: wk_sb[:, d_, :]
                else:
                    w_sl = lambda d_: wv_sb[:, d_, :]

                prjs = []
                for lc in range(NLC):
                    prj = ppr.tile([P, 512], _F32, tag="prj", bufs=4,
                                   name=f"prj_{et}_{lc}")
                    prjs.append(prj)
                for dti in range(NDT):
                    for lc in range(NLC):
                        nc.tensor.matmul(
                            prjs[lc][:], w_sl(dti),
                            xT[:, dti, lc * 512:(lc + 1) * 512],
                            start=(dti == 0), stop=(dti == NDT - 1))

                for lc in range(NLC):
                    ls = slice(lc * 512, (lc + 1) * 512)
                    qs = wsb.tile([P, 512], _BF, tag="qs", name=f"qs_{et}_{lc}")
                    nc.vector.tensor_copy(qs[:], prjs[lc][:])
                    if et < 5:
                        qrot = ppr.tile([P, 512], _F32, tag="qrot", bufs=2,
                                        name=f"qrot_{et}_{lc}")
                        nc.tensor.matmul(qrot[:], perm_sb[:], qs[:],
                                         start=True, stop=True)
                        tt = wsb.tile([P, 512], _BF, tag="tt",
                                      name=f"tt_{et}_{lc}")
                        nc.vector.tensor_mul(tt[:], qs[:], cos_sb[:, ls])
                        uu = wsb.tile([P, 512], _BF, tag="uu",
                                      name=f"uu_{et}_{lc}")
                        nc.vector.tensor_mul(uu[:], qrot[:], sin_sb[:, ls])
                        dest = qT[:, et, ls] if et < 4 else kT[:, ls]
                        nc.vector.tensor_add(dest, tt[:], uu[:])
                    else:
                        vtp = ppr.tile([P, 512], _BF, tag="vtp", bufs=2,
                                       name=f"vtp_{lc}")
                        for j in range(4):
                            nc.tensor.matmul(
                                vtp[:, j * P:(j + 1) * P],
                                qs[:, j * P:(j + 1) * P], id_sb[:],
                                is_transpose=True, skip_group_check=True)
                        nc.vector.tensor_copy(
                            vn[:, lc * 4:lc * 4 + 4, :],
                            vtp[:].rearrange("p (a b) -> p a b", a=4))

        # ---- attention + output projection
        with tc.tile_pool(name="pat", bufs=1, space="PSUM") as pat, \
             tc.tile_pool(name="pyp", bufs=1, space="PSUM") as pyp:
            for h in range(NHL):
                for qi in range(NLC):
                    q0 = qi * 512
                    nvis = q0 // P
                    nkt = nvis + 4
                    psum_o = pat.tile([P, 512], _F32, tag="po", bufs=2,
                                      name=f"po_{h}_{qi}")
                    psum_sum = pat.tile([P, 512], _F32, tag="ps", bufs=2,
                                        name=f"ps_{h}_{qi}")
                    for kt in range(nkt):
                        off = max(0, (kt - nvis) * P)
                        cs = slice(off, 512)
                        psum_s = pat.tile([P, 512], _F32, tag="sc", bufs=2,
                                          name=f"sc_{h}_{qi}_{kt}")
                        nc.tensor.matmul(
                            psum_s[:, cs], kT[:, kt * P:(kt + 1) * P],
                            qT[:, h, q0 + off:q0 + 512],
                            start=True, stop=True, skip_group_check=True)
                        es = wsb.tile([P, 512], _BF, tag="es", bufs=4,
                                      name=f"es_{h}_{qi}_{kt}")
                        nc.scalar.activation(
                            es[:, cs], psum_s[:, cs],
                            mybir.ActivationFunctionType.Exp, scale=SM_SCALE)
                        if kt >= nvis:
                            nc.vector.tensor_mul(es[:, off:off + P],
                                                 es[:, off:off + P], tri_sb[:])
                        nc.tensor.matmul(
                            psum_sum[:, cs], ones_sb[:], es[:, cs],
                            start=(kt == 0), stop=(kt == nkt - 1),
                            skip_group_check=True)
                        nc.tensor.matmul(
                            psum_o[:, cs], vn[:, kt, :], es[:, cs],
                            start=(kt == 0), stop=(kt == nkt - 1),
                            skip_group_check=True)
                    rec = wsb.tile([P, 512], _F32, tag="rec", bufs=2,
                                   name=f"rec_{h}_{qi}")
                    nc.vector.reciprocal(rec[:], psum_sum[:])
                    nc.vector.tensor_mul(oT[:, h, q0:q0 + 512],
                                         psum_o[:], rec[:])

            for lt in range(NKT):
                for mp in range(2):           # pairs of 512-wide out chunks
                    pys = []
                    for mi in range(2):
                        py_t = pyp.tile([P, 512], _F32, tag=f"py{mi}", bufs=1,
                                        name=f"py_{lt}_{mp}_{mi}")
                        pys.append(py_t)
                    for h in range(NHL):
                        for mi in range(2):
                            mc = mp * 2 + mi
                            nc.tensor.matmul(
                                pys[mi][:], oT[:, h, lt * P:(lt + 1) * P],
                                wo_sb[:, h, mc * 512:(mc + 1) * 512],
                                start=(h == 0), stop=(h == NHL - 1))
                    for mi in range(2):
                        mc = mp * 2 + mi
                        ysb = wsb.tile([P, 512], _F32, tag="ysb", bufs=3,
                                       name=f"ysb_{lt}_{mc}")
                        nc.vector.tensor_copy(ysb[:], pys[mi][:])
                        nc.sync.dma_start(
                            y_d[lt * P:(lt + 1) * P, mc * 512:(mc + 1) * 512],
                            ysb[:])


def host_constants():
    inv = (1.0 / (10000.0 ** (np.arange(0, HD, 2, dtype=np.float32) / HD))
           ).astype(np.float32)
    t = np.arange(L, dtype=np.float32)
    freqs = t[:, None] * inv[None, :]                    # [L, 64]
    emb = np.concatenate([freqs, freqs], axis=-1)        # [L, 128]
    cosT = np.ascontiguousarray(np.cos(emb).T).astype(BF16)
    sinT = np.ascontiguousarray(np.sin(emb).T).astype(BF16)
    perm = np.zeros((P, P), dtype=F32)
    for i in range(64):
        perm[i + 64, i] = -1.0      # qrot[d] = -q[d+64],  d < 64
        perm[i, i + 64] = 1.0       # qrot[d] =  q[d-64],  d >= 64
    ones = np.ones((P, P), dtype=F32)
    tri = (np.arange(P)[:, None] <= np.arange(P)[None, :]).astype(F32)  # k<=q
    ident = np.eye(P, dtype=F32)
    return {
        "cosT": cosT, "sinT": sinT,
        "perm": perm.astype(BF16), "ones": ones.astype(BF16),
        "tri": tri.astype(BF16), "ident": ident.astype(BF16),
    }


def make_in_map(consts, x, Wq, Wk, Wv, Wo, b, g):
    qs = slice(g * 512, (g + 1) * 512)
    kvs = slice(g * 128, (g + 1) * 128)
    wq = np.ascontiguousarray(
        Wq[qs].T.reshape(NDT, P, 512).transpose(1, 0, 2)).astype(BF16)
    wk = np.ascontiguousarray(
        Wk[kvs].T.reshape(NDT, P, 128).transpose(1, 0, 2)).astype(BF16)
    wv = np.ascontiguousarray(
        Wv[kvs].T.reshape(NDT, P, 128).transpose(1, 0, 2)).astype(BF16)
    wo = np.ascontiguousarray(
        Wo[:, qs].T.reshape(NHL, P, D).transpose(1, 0, 2)).astype(BF16)
    return {
        "x": np.ascontiguousarray(x[b]).astype(BF16),
        "wq": wq, "wk": wk, "wv": wv, "wo": wo,
        **consts,
    }


_NC_CACHE = {}


def get_nc():
    if "nc" not in _NC_CACHE:
        _NC_CACHE["nc"] = build_nc()
    return _NC_CACHE["nc"]


def kernel(x, Wq, Wk, Wv, Wo):
    x = np.asarray(x, dtype=F32)
    Wq = np.asarray(Wq, dtype=F32)
    Wk = np.asarray(Wk, dtype=F32)
    Wv = np.asarray(Wv, dtype=F32)
    Wo = np.asarray(Wo, dtype=F32)
    nc = get_nc()
    consts = host_constants()
    in_maps = [make_in_map(consts, x, Wq, Wk, Wv, Wo, c // 4, c % 4)
               for c in range(8)]
    res = run_bass_kernel_spmd(nc, in_maps, list(range(8)))
    outs = [r["y"].astype(np.float64) for r in res.results]
    y = np.stack([sum(outs[0:4]), sum(outs[4:8])], axis=0).astype(F32)
    return y



# revision 7
# speedup vs baseline: 1.1940x; 1.1940x over previous
"""Causal GQA self-attention (RoPE) Trainium2 Bass kernel, 8-core SPMD.

Sharding: core c -> (b = c//4, g = c%4).  Data-parallel over batch B=2,
tensor-parallel over the 4 KV groups (4 query heads + 1 KV head each).
Each core computes a partial output y_bg = attn_out_g @ Wo[:, g-block].T
for its batch; the host sums the 4 group partials per batch (row-parallel
linear unshard).

v2 layout (all matmuls bf16, f32 PSUM accumulation):
  xT is transposed on the HOST and DMA'd as [128, 16, L] bf16 (no PE
  transposes).  DMA issue order matches compute order so the PE starts
  ~6us in.  Projection chunks (512 queries) and attention chunks are
  interleaved so the ACT engine's exp work overlaps projection matmuls.
  Attention runs a 2-head, lookahead-2 software pipeline per chunk:
  S-matmul groups run two key-tiles ahead of their exp consumers, which
  hides the ACT exp latency that dominated the v1 stalls.  The causal
  mask is applied with a PE matmul (identity x mask-const accumulated
  into PSUM before the S matmul) instead of a DVE multiply, keeping the
  softmax critical path PE->ACT->PE only.  Output projection runs last
  with 4 rotating PSUM banks; PSUM->SBUF copies run on the ACT engine
  and y tiles stream out per 128x512 block.
"""

import math
import sys

import numpy as np

try:
    import concourse.bass as bass  # noqa: F401
except ImportError:  # pragma: no cover
    sys.path.insert(0, "/opt/trn_rl_repo")
    import concourse.bass as bass  # noqa: F401

import ml_dtypes

import concourse.bacc as bacc
import concourse.mybir as mybir
import concourse.tile as tile
from concourse.bass_utils import run_bass_kernel_spmd

BF16 = ml_dtypes.bfloat16
F32 = np.float32

B, L, D = 2, 2048, 2048
HD = 128          # head dim
NHL = 4           # query heads per core (one KV group)
P = 128
NDT = D // P      # 16 d-tiles
NKT = L // P      # 16 key tiles
NLC = L // 512    # 4 512-wide l chunks
SM_SCALE = 1.0 / math.sqrt(HD)
MASK_NEG = -30000.0

_BF = mybir.dt.bfloat16
_F32 = mybir.dt.float32
_EXP = mybir.ActivationFunctionType.Exp
_COPY = mybir.ActivationFunctionType.Copy


def build_nc():
    nc = bacc.Bacc("TRN2", target_bir_lowering=False, debug=False,
                   enable_asserts=False)

    xT_d = nc.dram_tensor("xT", [P, NDT, L], _BF, kind="ExternalInput").ap()
    wq_d = nc.dram_tensor("wq", [P, NHL, NDT, 128], _BF,
                          kind="ExternalInput").ap()
    wk_d = nc.dram_tensor("wk", [P, NDT, 128], _BF, kind="ExternalInput").ap()
    wv_d = nc.dram_tensor("wv", [P, NDT, 128], _BF, kind="ExternalInput").ap()
    wo_d = nc.dram_tensor("wo", [P, NHL, L], _BF, kind="ExternalInput").ap()
    cos_d = nc.dram_tensor("cosT", [P, L], _BF, kind="ExternalInput").ap()
    sin_d = nc.dram_tensor("sinT", [P, L], _BF, kind="ExternalInput").ap()
    perm_d = nc.dram_tensor("perm", [P, P], _BF, kind="ExternalInput").ap()
    ones_d = nc.dram_tensor("ones", [P, P], _BF, kind="ExternalInput").ap()
    msk_d = nc.dram_tensor("msk", [P, P], _BF, kind="ExternalInput").ap()
    id_d = nc.dram_tensor("ident", [P, P], _BF, kind="ExternalInput").ap()
    y_d = nc.dram_tensor("y", [L, D], _F32, kind="ExternalOutput").ap()

    with tile.TileContext(nc) as tc:
        _body(nc, tc, xT_d, wq_d, wk_d, wv_d, wo_d, cos_d, sin_d,
              perm_d, ones_d, msk_d, id_d, y_d)
    nc.compile()
    return nc


def _proj_segment(nc, tc, pa, wsb, lc, xT, wq_sb, wk_sb, wv_sb,
                  cos_sb, sin_sb, perm_sb, id_sb, qT, kT, vn):
    """Q/K/V projections + RoPE for one 512-query chunk."""
    ls = slice(lc * 512, (lc + 1) * 512)
    for et in (4, 5, 0, 1, 2, 3):          # k, v, then the 4 q heads
        if et < 4:
            w_sl = lambda d_: wq_sb[:, et, d_, :]
        elif et == 4:
            w_sl = lambda d_: wk_sb[:, d_, :]
        else:
            w_sl = lambda d_: wv_sb[:, d_, :]

        prj = pa.tile([P, 512], _F32, tag="prj", bufs=5,
                      name=f"prj_{lc}_{et}")
        for dti in range(NDT):
            nc.tensor.matmul(prj[:], w_sl(dti), xT[:, dti, ls],
                             start=(dti == 0), stop=(dti == NDT - 1))
        qs = wsb.tile([P, 512], _BF, tag="qs", bufs=3, name=f"qs_{lc}_{et}")
        nc.vector.tensor_copy(qs[:], prj[:])
        if et == 5:
            vtp = pa.tile([P, 512], _BF, tag="vtp", bufs=1,
                          name=f"vtp_{lc}")
            for j in range(4):
                nc.tensor.matmul(vtp[:, j * P:(j + 1) * P],
                                 qs[:, j * P:(j + 1) * P], id_sb[:],
                                 is_transpose=True, skip_group_check=True)
            nc.vector.tensor_copy(vn[:, lc * 4:lc * 4 + 4, :],
                                  vtp[:].rearrange("p (a b) -> p a b", a=4))
        else:
            qrot = pa.tile([P, 512], _F32, tag="qrot", bufs=2,
                           name=f"qrot_{lc}_{et}")
            nc.tensor.matmul(qrot[:], perm_sb[:], qs[:], start=True,
                             stop=True)
            tt = wsb.tile([P, 512], _BF, tag="tt", bufs=2,
                          name=f"tt_{lc}_{et}")
            nc.vector.tensor_mul(tt[:], qs[:], cos_sb[:, ls])
            uu = wsb.tile([P, 512], _BF, tag="uu", bufs=2,
                          name=f"uu_{lc}_{et}")
            nc.vector.tensor_mul(uu[:], qrot[:], sin_sb[:, ls])
            dest = qT[:, et, ls] if et < 4 else kT[:, ls]
            nc.vector.tensor_add(dest, tt[:], uu[:])


def _attn_chunk(nc, tc, pb, wsb, qi, qT, kT, vn, oT, ones_sb, msk_sb,
                id_sb):
    """Causal attention for one 512-query chunk, all 4 heads.

    Two heads run in a lookahead-2 software pipeline over key tiles so
    the PE never waits on the ACT exp of the tile it is about to
    consume.
    """
    q0 = qi * 512
    nvis = 4 * qi
    nkt = nvis + 4

    for pair in ((0, 1), (2, 3)):
        po = {}
        ps = {}
        es = {}
        for h in pair:
            po[h] = pb.tile([P, 512], _F32, tag="po", bufs=2,
                            name=f"po_{qi}_{h}")
            ps[h] = pb.tile([P, 512], _F32, tag="ps", bufs=2,
                            name=f"ps_{qi}_{h}")

        def emit_s(h, kt):
            off = max(0, (kt - nvis) * P)
            cs = slice(off, 512)
            sc = pb.tile([P, 512], _F32, tag="sc", bufs=4,
                         name=f"sc_{qi}_{h}_{kt}")
            ktile = kT[:, kt * P:(kt + 1) * P]
            qtile = lambda o: qT[:, h, q0 + o:q0 + 512]
            if kt >= nvis:
                # diagonal tile: mask const first, S accumulates on top
                nc.tensor.matmul(sc[:, off:off + P], id_sb[:], msk_sb[:],
                                 start=True, stop=False,
                                 skip_group_check=True)
                nc.tensor.matmul(sc[:, off:off + P], ktile,
                                 qT[:, h, q0 + off:q0 + off + P],
                                 start=False, stop=True,
                                 skip_group_check=True)
                if off + P < 512:
                    nc.tensor.matmul(sc[:, off + P:512], ktile,
                                     qtile(off + P), start=True, stop=True,
                                     skip_group_check=True)
            else:
                nc.tensor.matmul(sc[:, cs], ktile, qtile(off),
                                 start=True, stop=True,
                                 skip_group_check=True)
            e = wsb.tile([P, 512], _BF, tag="es", bufs=8,
                         name=f"es_{qi}_{h}_{kt}")
            nc.scalar.activation(e[:, cs], sc[:, cs], _EXP, scale=SM_SCALE)
            es[(h, kt)] = e

        def emit_c(h, kt):
            off = max(0, (kt - nvis) * P)
            cs = slice(off, 512)
            e = es.pop((h, kt))
            nc.tensor.matmul(ps[h][:, cs], ones_sb[:], e[:, cs],
                             start=(kt == 0), stop=(kt == nkt - 1),
                             skip_group_check=True)
            nc.tensor.matmul(po[h][:, cs], vn[:, kt, :], e[:, cs],
                             start=(kt == 0), stop=(kt == nkt - 1),
                             skip_group_check=True)

        for kt in range(nkt + 2):
            if kt < nkt:
                for h in pair:
                    emit_s(h, kt)
            if kt >= 2:
                for h in pair:
                    emit_c(h, kt - 2)

        for h in pair:
            rec = wsb.tile([P, 512], _F32, tag="rec", bufs=2,
                           name=f"rec_{qi}_{h}")
            nc.vector.reciprocal(rec[:], ps[h][:])
            nc.vector.tensor_mul(oT[:, h, q0:q0 + 512], po[h][:], rec[:])


def _body(nc, tc, xT_d, wq_d, wk_d, wv_d, wo_d, cos_d, sin_d,
          perm_d, ones_d, msk_d, id_d, y_d):
    from contextlib import ExitStack
    ctx = ExitStack()
    with ctx:
        pp = ctx.enter_context(tc.tile_pool(name="persist", bufs=1))
        wsb = ctx.enter_context(tc.tile_pool(name="wsb", bufs=2))

        xT = pp.tile([P, NDT, L], _BF, tag="xT")
        wq_sb = pp.tile([P, NHL, NDT, 128], _BF, tag="wq")
        wk_sb = pp.tile([P, NDT, 128], _BF, tag="wk")
        wv_sb = pp.tile([P, NDT, 128], _BF, tag="wv")
        wo_sb = pp.tile([P, NHL, L], _BF, tag="wo")
        cos_sb = pp.tile([P, L], _BF, tag="cos")
        sin_sb = pp.tile([P, L], _BF, tag="sin")
        perm_sb = pp.tile([P, P], _BF, tag="perm")
        ones_sb = pp.tile([P, P], _BF, tag="ones")
        msk_sb = pp.tile([P, P], _BF, tag="msk")
        id_sb = pp.tile([P, P], _BF, tag="ident")
        qT = pp.tile([P, NHL, L], _BF, tag="qT")
        kT = pp.tile([P, L], _BF, tag="kT")
        vn = pp.tile([P, NKT, 128], _BF, tag="vn")
        oT = pp.tile([P, NHL, L], _BF, tag="oT")

        # DMA issue order tracks compute order (sync-engine DMAs are
        # FIFO and hold the SP sequencer while waiting on data).
        nc.sync.dma_start(id_sb[:], id_d[:])
        nc.sync.dma_start(msk_sb[:], msk_d[:])
        nc.sync.dma_start(perm_sb[:], perm_d[:])
        nc.sync.dma_start(ones_sb[:], ones_d[:])
        nc.sync.dma_start(wk_sb[:], wk_d[:])
        nc.sync.dma_start(xT[:, 0:8, 0:512], xT_d[:, 0:8, 0:512])
        nc.sync.dma_start(xT[:, 8:16, 0:512], xT_d[:, 8:16, 0:512])
        nc.sync.dma_start(wv_sb[:], wv_d[:])
        nc.sync.dma_start(cos_sb[:], cos_d[:])
        nc.sync.dma_start(sin_sb[:], sin_d[:])
        for hq in range(4):
            nc.sync.dma_start(wq_sb[:, hq], wq_d[:, hq])
        for lc in range(1, NLC):
            ls = slice(lc * 512, (lc + 1) * 512)
            nc.sync.dma_start(xT[:, :, ls], xT_d[:, :, ls])
        nc.sync.dma_start(wo_sb[:], wo_d[:])

        # interleaved projection / attention chunks
        for lc in range(NLC):
            with tc.tile_pool(name=f"pa{lc}", bufs=1, space="PSUM") as pa:
                _proj_segment(nc, tc, pa, wsb, lc, xT, wq_sb, wk_sb,
                              wv_sb, cos_sb, sin_sb, perm_sb, id_sb,
                              qT, kT, vn)
            with tc.tile_pool(name=f"pb{lc}", bufs=1, space="PSUM") as pb:
                _attn_chunk(nc, tc, pb, wsb, lc, qT, kT, vn, oT,
                            ones_sb, msk_sb, id_sb)

        # output projection, streamed out per 128x512 tile
        with tc.tile_pool(name="pc", bufs=1, space="PSUM") as pc:
            for lt in range(NKT):
                for mc in range(4):
                    py = pc.tile([P, 512], _F32, tag="py", bufs=4,
                                 name=f"py_{lt}_{mc}")
                    for h in range(NHL):
                        nc.tensor.matmul(
                            py[:], oT[:, h, lt * P:(lt + 1) * P],
                            wo_sb[:, h, mc * 512:(mc + 1) * 512],
                            start=(h == 0), stop=(h == NHL - 1))
                    ysb = wsb.tile([P, 512], _F32, tag="ysb", bufs=4,
                                   name=f"ysb_{lt}_{mc}")
                    nc.scalar.activation(ysb[:], py[:], _COPY)
                    nc.sync.dma_start(
                        y_d[lt * P:(lt + 1) * P, mc * 512:(mc + 1) * 512],
                        ysb[:])


def host_constants():
    inv = (1.0 / (10000.0 ** (np.arange(0, HD, 2, dtype=np.float32) / HD))
           ).astype(np.float32)
    t = np.arange(L, dtype=np.float32)
    freqs = t[:, None] * inv[None, :]                    # [L, 64]
    emb = np.concatenate([freqs, freqs], axis=-1)        # [L, 128]
    cosT = np.ascontiguousarray(np.cos(emb).T).astype(BF16)
    sinT = np.ascontiguousarray(np.sin(emb).T).astype(BF16)
    perm = np.zeros((P, P), dtype=F32)
    for i in range(64):
        perm[i + 64, i] = -1.0      # qrot[d] = -q[d+64],  d < 64
        perm[i, i + 64] = 1.0       # qrot[d] =  q[d-64],  d >= 64
    ones = np.ones((P, P), dtype=F32)
    # msk[k, t] = MASK_NEG where key k > query t (strict upper part per
    # diagonal 128-block); added into PSUM before the S matmul.
    msk = np.where(np.arange(P)[:, None] > np.arange(P)[None, :],
                   MASK_NEG, 0.0).astype(F32)
    ident = np.eye(P, dtype=F32)
    return {
        "cosT": cosT, "sinT": sinT,
        "perm": perm.astype(BF16), "ones": ones.astype(BF16),
        "msk": msk.astype(BF16), "ident": ident.astype(BF16),
    }


def make_in_map(consts, x, Wq, Wk, Wv, Wo, b, g):
    qs = slice(g * 512, (g + 1) * 512)
    kvs = slice(g * 128, (g + 1) * 128)
    wq = np.ascontiguousarray(
        Wq[qs].T.reshape(NDT, P, NHL, 128).transpose(1, 2, 0, 3)
    ).astype(BF16)
    wk = np.ascontiguousarray(
        Wk[kvs].T.reshape(NDT, P, 128).transpose(1, 0, 2)).astype(BF16)
    wv = np.ascontiguousarray(
        Wv[kvs].T.reshape(NDT, P, 128).transpose(1, 0, 2)).astype(BF16)
    wo = np.ascontiguousarray(
        Wo[:, qs].T.reshape(NHL, P, D).transpose(1, 0, 2)).astype(BF16)
    xT = np.ascontiguousarray(
        x[b].T.reshape(NDT, P, L).transpose(1, 0, 2)).astype(BF16)
    return {
        "xT": xT,
        "wq": wq, "wk": wk, "wv": wv, "wo": wo,
        **consts,
    }


_NC_CACHE = {}


def get_nc():
    if "nc" not in _NC_CACHE:
        _NC_CACHE["nc"] = build_nc()
    return _NC_CACHE["nc"]


def kernel(x, Wq, Wk, Wv, Wo):
    x = np.asarray(x, dtype=F32)
    Wq = np.asarray(Wq, dtype=F32)
    Wk = np.asarray(Wk, dtype=F32)
    Wv = np.asarray(Wv, dtype=F32)
    Wo = np.asarray(Wo, dtype=F32)
    nc = get_nc()
    consts = host_constants()
    in_maps = [make_in_map(consts, x, Wq, Wk, Wv, Wo, c // 4, c % 4)
               for c in range(8)]
    res = run_bass_kernel_spmd(nc, in_maps, list(range(8)))
    outs = [r["y"].astype(np.float64) for r in res.results]
    y = np.stack([sum(outs[0:4]), sum(outs[4:8])], axis=0).astype(F32)
    return y


# revision 11
# speedup vs baseline: 1.2189x; 1.0208x over previous
"""Causal GQA self-attention (RoPE) Trainium2 Bass kernel, 8-core SPMD.

Sharding: core c -> (b = c//4, g = c%4).  Data-parallel over batch B=2,
tensor-parallel over the 4 KV groups (4 query heads + 1 KV head each).
Each core computes a partial output y_bg = attn_out_g @ Wo[:, g-block].T
for its batch; the host sums the 4 group partials per batch (row-parallel
linear unshard).

v2 layout (all matmuls bf16, f32 PSUM accumulation):
  xT is transposed on the HOST and DMA'd as [128, 16, L] bf16 (no PE
  transposes).  DMA issue order matches compute order so the PE starts
  ~6us in.  Projection chunks (512 queries) and attention chunks are
  interleaved so the ACT engine's exp work overlaps projection matmuls.
  Attention runs a 2-head, lookahead-2 software pipeline per chunk:
  S-matmul groups run two key-tiles ahead of their exp consumers, which
  hides the ACT exp latency that dominated the v1 stalls.  The causal
  mask is applied with a PE matmul (identity x mask-const accumulated
  into PSUM before the S matmul) instead of a DVE multiply, keeping the
  softmax critical path PE->ACT->PE only.  Output projection runs last
  with 4 rotating PSUM banks; PSUM->SBUF copies run on the ACT engine
  and y tiles stream out per 128x512 block.
"""

import math
import sys

import numpy as np

try:
    import concourse.bass as bass  # noqa: F401
except ImportError:  # pragma: no cover
    sys.path.insert(0, "/opt/trn_rl_repo")
    import concourse.bass as bass  # noqa: F401

import ml_dtypes

import concourse.bacc as bacc
import concourse.mybir as mybir
import concourse.tile as tile
from concourse.bass_utils import run_bass_kernel_spmd

BF16 = ml_dtypes.bfloat16
F32 = np.float32

B, L, D = 2, 2048, 2048
HD = 128          # head dim
NHL = 4           # query heads per core (one KV group)
P = 128
NDT = D // P      # 16 d-tiles
NKT = L // P      # 16 key tiles
NLC = L // 512    # 4 512-wide l chunks
SM_SCALE = 1.0 / math.sqrt(HD)
MASK_NEG = -30000.0

_BF = mybir.dt.bfloat16
_F32 = mybir.dt.float32
_EXP = mybir.ActivationFunctionType.Exp
_COPY = mybir.ActivationFunctionType.Copy


def build_nc():
    nc = bacc.Bacc("TRN2", target_bir_lowering=False, debug=False,
                   enable_asserts=False)

    xT_d = nc.dram_tensor("xT", [P, NDT, L], _BF, kind="ExternalInput").ap()
    wq_d = nc.dram_tensor("wq", [P, NHL, NDT, 128], _BF,
                          kind="ExternalInput").ap()
    wk_d = nc.dram_tensor("wk", [P, NDT, 128], _BF, kind="ExternalInput").ap()
    wv_d = nc.dram_tensor("wv", [P, NDT, 128], _BF, kind="ExternalInput").ap()
    wo_d = nc.dram_tensor("wo", [P, NHL, L], _BF, kind="ExternalInput").ap()
    cos_d = nc.dram_tensor("cosT", [P, L], _BF, kind="ExternalInput").ap()
    sin_d = nc.dram_tensor("sinT", [P, L], _BF, kind="ExternalInput").ap()
    perm_d = nc.dram_tensor("perm", [P, P], _BF, kind="ExternalInput").ap()
    ones_d = nc.dram_tensor("ones", [P, P], _BF, kind="ExternalInput").ap()
    msk_d = nc.dram_tensor("msk", [P, P], _BF, kind="ExternalInput").ap()
    id_d = nc.dram_tensor("ident", [P, P], _BF, kind="ExternalInput").ap()
    y_d = nc.dram_tensor("y", [L, D], _F32, kind="ExternalOutput").ap()

    with tile.TileContext(nc) as tc:
        _body(nc, tc, xT_d, wq_d, wk_d, wv_d, wo_d, cos_d, sin_d,
              perm_d, ones_d, msk_d, id_d, y_d)
    nc.compile()
    return nc


def _proj_segment(nc, tc, pa, wsb, lc, xT, wq_sb, wk_sb, wv_sb,
                  cos_sb, sin_sb, perm_sb, id_sb, qT, kT, vn):
    """Q/K/V projections + RoPE for one 512-query chunk."""
    ls = slice(lc * 512, (lc + 1) * 512)
    for et in (4, 5, 0, 1, 2, 3):          # k, v, then the 4 q heads
        if et < 4:
            w_sl = lambda d_: wq_sb[:, et, d_, :]
        elif et == 4:
            w_sl = lambda d_: wk_sb[:, d_, :]
        else:
            w_sl = lambda d_: wv_sb[:, d_, :]

        prj = pa.tile([P, 512], _F32, tag="prj", bufs=5,
                      name=f"prj_{lc}_{et}")
        for dti in range(NDT):
            nc.tensor.matmul(prj[:], w_sl(dti), xT[:, dti, ls],
                             start=(dti == 0), stop=(dti == NDT - 1))
        qs = wsb.tile([P, 512], _BF, tag="qs", bufs=3, name=f"qs_{lc}_{et}")
        nc.vector.tensor_copy(qs[:], prj[:])
        if et == 5:
            vtp = pa.tile([P, 512], _BF, tag="vtp", bufs=1,
                          name=f"vtp_{lc}")
            for j in range(4):
                nc.tensor.matmul(vtp[:, j * P:(j + 1) * P],
                                 qs[:, j * P:(j + 1) * P], id_sb[:],
                                 is_transpose=True, skip_group_check=True)
            nc.vector.tensor_copy(vn[:, lc * 4:lc * 4 + 4, :],
                                  vtp[:].rearrange("p (a b) -> p a b", a=4))
        else:
            qrot = pa.tile([P, 512], _F32, tag="qrot", bufs=2,
                           name=f"qrot_{lc}_{et}")
            nc.tensor.matmul(qrot[:], perm_sb[:], qs[:], start=True,
                             stop=True)
            tt = wsb.tile([P, 512], _BF, tag="tt", bufs=2,
                          name=f"tt_{lc}_{et}")
            nc.vector.tensor_mul(tt[:], qs[:], cos_sb[:, ls])
            uu = wsb.tile([P, 512], _BF, tag="uu", bufs=2,
                          name=f"uu_{lc}_{et}")
            nc.vector.tensor_mul(uu[:], qrot[:], sin_sb[:, ls])
            dest = qT[:, et, ls] if et < 4 else kT[:, ls]
            nc.vector.tensor_add(dest, tt[:], uu[:])


def _op_group(nc, pool, wsb, lt, mc, oT, wo_sb, y_d, bufs):
    """One output-projection PSUM group: 4 head-matmuls -> DVE copy -> DMA."""
    py = pool.tile([P, 512], _F32, tag="py", bufs=bufs,
                   name=f"py_{lt}_{mc}")
    for h in range(NHL):
        nc.tensor.matmul(py[:], oT[:, h, lt * P:(lt + 1) * P],
                         wo_sb[:, h, mc * 512:(mc + 1) * 512],
                         start=(h == 0), stop=(h == NHL - 1))
    ysb = wsb.tile([P, 512], _F32, tag="ysb", bufs=4, name=f"ysb_{lt}_{mc}")
    nc.vector.tensor_copy(ysb[:], py[:])
    nc.sync.dma_start(y_d[lt * P:(lt + 1) * P, mc * 512:(mc + 1) * 512],
                      ysb[:])


def _attn_chunk(nc, tc, pb, wsb, qi, qT, kT, vn, oT, ones_sb, msk_sb,
                id_sb, op_iter, op_args):
    """Causal attention for one 512-query chunk, all 4 heads.

    Two heads run in a software pipeline over key tiles so the PE never
    waits on the ACT exp of the tile it is about to consume.  When
    ``op_iter`` is set, one output-projection group of the previous
    chunk is interleaved per round as additional exp-latency cover
    (lookahead drops to 1 to fit PSUM: sc3+po2+ps2+py1 banks).
    """
    q0 = qi * 512
    nvis = 4 * qi
    nkt = nvis + 4
    look = 2 if op_iter is None else 1
    sc_bufs = 4 if op_iter is None else 3

    def emit_op():
        if op_iter is not None:
            nxt = next(op_iter, None)
            if nxt is not None:
                _op_group(nc, pb, wsb, nxt[0], nxt[1], *op_args, bufs=1)

    for pair in ((0, 1), (2, 3)):
        po = {}
        ps = {}
        es = {}
        for h in pair:
            po[h] = pb.tile([P, 512], _F32, tag="po", bufs=2,
                            name=f"po_{qi}_{h}")
            ps[h] = pb.tile([P, 512], _F32, tag="ps", bufs=2,
                            name=f"ps_{qi}_{h}")

        def emit_s(h, kt):
            off = max(0, (kt - nvis) * P)
            cs = slice(off, 512)
            sc = pb.tile([P, 512], _F32, tag="sc", bufs=sc_bufs,
                         name=f"sc_{qi}_{h}_{kt}")
            ktile = kT[:, kt * P:(kt + 1) * P]
            qtile = lambda o: qT[:, h, q0 + o:q0 + 512]
            if kt >= nvis:
                # diagonal tile: mask const first, S accumulates on top
                nc.tensor.matmul(sc[:, off:off + P], id_sb[:], msk_sb[:],
                                 start=True, stop=False,
                                 skip_group_check=True)
                nc.tensor.matmul(sc[:, off:off + P], ktile,
                                 qT[:, h, q0 + off:q0 + off + P],
                                 start=False, stop=True,
                                 skip_group_check=True)
                if off + P < 512:
                    nc.tensor.matmul(sc[:, off + P:512], ktile,
                                     qtile(off + P), start=True, stop=True,
                                     skip_group_check=True)
            else:
                nc.tensor.matmul(sc[:, cs], ktile, qtile(off),
                                 start=True, stop=True,
                                 skip_group_check=True)
            e = wsb.tile([P, 512], _BF, tag="es", bufs=8,
                         name=f"es_{qi}_{h}_{kt}")
            nc.scalar.activation(e[:, cs], sc[:, cs], _EXP, scale=SM_SCALE)
            es[(h, kt)] = e

        def emit_c(h, kt):
            off = max(0, (kt - nvis) * P)
            cs = slice(off, 512)
            e = es.pop((h, kt))
            nc.tensor.matmul(ps[h][:, cs], ones_sb[:], e[:, cs],
                             start=(kt == 0), stop=(kt == nkt - 1),
                             skip_group_check=True)
            nc.tensor.matmul(po[h][:, cs], vn[:, kt, :], e[:, cs],
                             start=(kt == 0), stop=(kt == nkt - 1),
                             skip_group_check=True)

        for kt in range(nkt + look):
            if kt < nkt:
                for h in pair:
                    emit_s(h, kt)
            emit_op()
            if kt >= look:
                for h in pair:
                    emit_c(h, kt - look)

        for h in pair:
            rec = wsb.tile([P, 512], _F32, tag="rec", bufs=2,
                           name=f"rec_{qi}_{h}")
            nc.vector.reciprocal(rec[:], ps[h][:])
            nc.vector.tensor_mul(oT[:, h, q0:q0 + 512], po[h][:], rec[:])


def _body(nc, tc, xT_d, wq_d, wk_d, wv_d, wo_d, cos_d, sin_d,
          perm_d, ones_d, msk_d, id_d, y_d):
    from contextlib import ExitStack
    ctx = ExitStack()
    with ctx:
        pp = ctx.enter_context(tc.tile_pool(name="persist", bufs=1))
        wsb = ctx.enter_context(tc.tile_pool(name="wsb", bufs=2))

        xT = pp.tile([P, NDT, L], _BF, tag="xT")
        wq_sb = pp.tile([P, NHL, NDT, 128], _BF, tag="wq")
        wk_sb = pp.tile([P, NDT, 128], _BF, tag="wk")
        wv_sb = pp.tile([P, NDT, 128], _BF, tag="wv")
        wo_sb = pp.tile([P, NHL, L], _BF, tag="wo")
        cos_sb = pp.tile([P, L], _BF, tag="cos")
        sin_sb = pp.tile([P, L], _BF, tag="sin")
        perm_sb = pp.tile([P, P], _BF, tag="perm")
        ones_sb = pp.tile([P, P], _BF, tag="ones")
        msk_sb = pp.tile([P, P], _BF, tag="msk")
        id_sb = pp.tile([P, P], _BF, tag="ident")
        qT = pp.tile([P, NHL, L], _BF, tag="qT")
        kT = pp.tile([P, L], _BF, tag="kT")
        vn = pp.tile([P, NKT, 128], _BF, tag="vn")
        oT = pp.tile([P, NHL, L], _BF, tag="oT")

        # DMA issue order tracks compute order (sync-engine DMAs are
        # FIFO and hold the SP sequencer while waiting on data).
        nc.sync.dma_start(id_sb[:], id_d[:])
        nc.sync.dma_start(msk_sb[:], msk_d[:])
        nc.sync.dma_start(perm_sb[:], perm_d[:])
        nc.sync.dma_start(ones_sb[:], ones_d[:])
        nc.sync.dma_start(wk_sb[:], wk_d[:])
        nc.sync.dma_start(xT[:, 0:8, 0:512], xT_d[:, 0:8, 0:512])
        nc.sync.dma_start(xT[:, 8:16, 0:512], xT_d[:, 8:16, 0:512])
        nc.sync.dma_start(wv_sb[:], wv_d[:])
        nc.sync.dma_start(cos_sb[:], cos_d[:])
        nc.sync.dma_start(sin_sb[:], sin_d[:])
        for hq in range(4):
            nc.sync.dma_start(wq_sb[:, hq], wq_d[:, hq])
        for lc in range(1, NLC):
            ls = slice(lc * 512, (lc + 1) * 512)
            nc.sync.dma_start(xT[:, :, ls], xT_d[:, :, ls])
        nc.sync.dma_start(wo_sb[:], wo_d[:])

        # interleaved projection / attention chunks; attention chunk qi
        # also consumes the output-projection groups of chunk qi-1
        op_args = (oT, wo_sb, y_d)
        for lc in range(NLC):
            with tc.tile_pool(name=f"pa{lc}", bufs=1, space="PSUM") as pa:
                _proj_segment(nc, tc, pa, wsb, lc, xT, wq_sb, wk_sb,
                              wv_sb, cos_sb, sin_sb, perm_sb, id_sb,
                              qT, kT, vn)
            op_iter = None
            if lc > 0:
                op_iter = iter([(lt, mc) for lt in range(4 * (lc - 1),
                                                        4 * lc)
                                for mc in range(4)])
            with tc.tile_pool(name=f"pb{lc}", bufs=1, space="PSUM") as pb:
                _attn_chunk(nc, tc, pb, wsb, lc, qT, kT, vn, oT,
                            ones_sb, msk_sb, id_sb, op_iter, op_args)

        # last chunk's output projection, streamed out per 128x512 tile
        with tc.tile_pool(name="pc", bufs=1, space="PSUM") as pc:
            for lt in range(12, 16):
                for mc in range(4):
                    _op_group(nc, pc, wsb, lt, mc, oT, wo_sb, y_d, bufs=4)


def host_constants():
    inv = (1.0 / (10000.0 ** (np.arange(0, HD, 2, dtype=np.float32) / HD))
           ).astype(np.float32)
    t = np.arange(L, dtype=np.float32)
    freqs = t[:, None] * inv[None, :]                    # [L, 64]
    emb = np.concatenate([freqs, freqs], axis=-1)        # [L, 128]
    cosT = np.ascontiguousarray(np.cos(emb).T).astype(BF16)
    sinT = np.ascontiguousarray(np.sin(emb).T).astype(BF16)
    perm = np.zeros((P, P), dtype=F32)
    for i in range(64):
        perm[i + 64, i] = -1.0      # qrot[d] = -q[d+64],  d < 64
        perm[i, i + 64] = 1.0       # qrot[d] =  q[d-64],  d >= 64
    ones = np.ones((P, P), dtype=F32)
    # msk[k, t] = MASK_NEG where key k > query t (strict upper part per
    # diagonal 128-block); added into PSUM before the S matmul.
    msk = np.where(np.arange(P)[:, None] > np.arange(P)[None, :],
                   MASK_NEG, 0.0).astype(F32)
    ident = np.eye(P, dtype=F32)
    return {
        "cosT": cosT, "sinT": sinT,
        "perm": perm.astype(BF16), "ones": ones.astype(BF16),
        "msk": msk.astype(BF16), "ident": ident.astype(BF16),
    }


def make_in_map(consts, x, Wq, Wk, Wv, Wo, b, g):
    qs = slice(g * 512, (g + 1) * 512)
    kvs = slice(g * 128, (g + 1) * 128)
    wq = np.ascontiguousarray(
        Wq[qs].T.reshape(NDT, P, NHL, 128).transpose(1, 2, 0, 3)
    ).astype(BF16)
    wk = np.ascontiguousarray(
        Wk[kvs].T.reshape(NDT, P, 128).transpose(1, 0, 2)).astype(BF16)
    wv = np.ascontiguousarray(
        Wv[kvs].T.reshape(NDT, P, 128).transpose(1, 0, 2)).astype(BF16)
    wo = np.ascontiguousarray(
        Wo[:, qs].T.reshape(NHL, P, D).transpose(1, 0, 2)).astype(BF16)
    xT = np.ascontiguousarray(
        x[b].T.reshape(NDT, P, L).transpose(1, 0, 2)).astype(BF16)
    return {
        "xT": xT,
        "wq": wq, "wk": wk, "wv": wv, "wo": wo,
        **consts,
    }


_NC_CACHE = {}


def get_nc():
    if "nc" not in _NC_CACHE:
        _NC_CACHE["nc"] = build_nc()
    return _NC_CACHE["nc"]


def kernel(x, Wq, Wk, Wv, Wo):
    x = np.asarray(x, dtype=F32)
    Wq = np.asarray(Wq, dtype=F32)
    Wk = np.asarray(Wk, dtype=F32)
    Wv = np.asarray(Wv, dtype=F32)
    Wo = np.asarray(Wo, dtype=F32)
    nc = get_nc()
    consts = host_constants()
    in_maps = [make_in_map(consts, x, Wq, Wk, Wv, Wo, c // 4, c % 4)
               for c in range(8)]
    res = run_bass_kernel_spmd(nc, in_maps, list(range(8)))
    outs = [r["y"].astype(np.float64) for r in res.results]
    y = np.stack([sum(outs[0:4]), sum(outs[4:8])], axis=0).astype(F32)
    return y


# revision 17
# speedup vs baseline: 1.2205x; 1.0013x over previous
"""Causal GQA self-attention (RoPE) Trainium2 Bass kernel, 8-core SPMD.

Sharding: core c -> (b = c//4, g = c%4).  Data-parallel over batch B=2,
tensor-parallel over the 4 KV groups (4 query heads + 1 KV head each).
Each core computes a partial output y_bg = attn_out_g @ Wo[:, g-block].T
for its batch; the host sums the 4 group partials per batch (row-parallel
linear unshard).

v2 layout (all matmuls bf16, f32 PSUM accumulation):
  xT is transposed on the HOST and DMA'd as [128, 16, L] bf16 (no PE
  transposes).  DMA issue order matches compute order so the PE starts
  ~6us in.  Projection chunks (512 queries) and attention chunks are
  interleaved so the ACT engine's exp work overlaps projection matmuls.
  Attention runs a 2-head, lookahead-2 software pipeline per chunk:
  S-matmul groups run two key-tiles ahead of their exp consumers, which
  hides the ACT exp latency that dominated the v1 stalls.  The causal
  mask is applied with a PE matmul (identity x mask-const accumulated
  into PSUM before the S matmul) instead of a DVE multiply, keeping the
  softmax critical path PE->ACT->PE only.  Output projection runs last
  with 4 rotating PSUM banks; PSUM->SBUF copies run on the ACT engine
  and y tiles stream out per 128x512 block.
"""

import math
import sys

import numpy as np

try:
    import concourse.bass as bass  # noqa: F401
except ImportError:  # pragma: no cover
    sys.path.insert(0, "/opt/trn_rl_repo")
    import concourse.bass as bass  # noqa: F401

import ml_dtypes

import concourse.bacc as bacc
import concourse.mybir as mybir
import concourse.tile as tile
from concourse.bass_utils import run_bass_kernel_spmd

BF16 = ml_dtypes.bfloat16
F32 = np.float32

B, L, D = 2, 2048, 2048
HD = 128          # head dim
NHL = 4           # query heads per core (one KV group)
P = 128
NDT = D // P      # 16 d-tiles
NKT = L // P      # 16 key tiles
NLC = L // 512    # 4 512-wide l chunks
SM_SCALE = 1.0 / math.sqrt(HD)
MASK_NEG = -30000.0

_BF = mybir.dt.bfloat16
_F32 = mybir.dt.float32
_EXP = mybir.ActivationFunctionType.Exp
_COPY = mybir.ActivationFunctionType.Copy


def build_nc():
    nc = bacc.Bacc("TRN2", target_bir_lowering=False, debug=False,
                   enable_asserts=False)

    xT_d = nc.dram_tensor("xT", [P, NDT, L], _BF, kind="ExternalInput").ap()
    wq_d = nc.dram_tensor("wq", [P, NHL, NDT, 128], _BF,
                          kind="ExternalInput").ap()
    wk_d = nc.dram_tensor("wk", [P, NDT, 128], _BF, kind="ExternalInput").ap()
    wv_d = nc.dram_tensor("wv", [P, NDT, 128], _BF, kind="ExternalInput").ap()
    wo_d = nc.dram_tensor("wo", [P, NHL, L], _BF, kind="ExternalInput").ap()
    cos_d = nc.dram_tensor("cosT", [P, L], _BF, kind="ExternalInput").ap()
    sin_d = nc.dram_tensor("sinT", [P, L], _BF, kind="ExternalInput").ap()
    perm_d = nc.dram_tensor("perm", [P, P], _BF, kind="ExternalInput").ap()
    ones_d = nc.dram_tensor("ones", [P, P], _BF, kind="ExternalInput").ap()
    msk_d = nc.dram_tensor("msk", [P, P], _BF, kind="ExternalInput").ap()
    id_d = nc.dram_tensor("ident", [P, P], _BF, kind="ExternalInput").ap()
    y_d = nc.dram_tensor("y", [L, D], _F32, kind="ExternalOutput").ap()

    with tile.TileContext(nc) as tc:
        _body(nc, tc, xT_d, wq_d, wk_d, wv_d, wo_d, cos_d, sin_d,
              perm_d, ones_d, msk_d, id_d, y_d)
    nc.compile()
    return nc


def _proj_segment(nc, tc, pa, wsb, lc, xT, wq_sb, wk_sb, wv_sb,
                  cos_sb, sin_sb, perm_sb, id_sb, qT, kT, vn):
    """Q/K/V projections + RoPE for one 512-query chunk."""
    ls = slice(lc * 512, (lc + 1) * 512)
    for et in (4, 5, 0, 1, 2, 3):          # k, v, then the 4 q heads
        if et < 4:
            w_sl = lambda d_: wq_sb[:, et, d_, :]
        elif et == 4:
            w_sl = lambda d_: wk_sb[:, d_, :]
        else:
            w_sl = lambda d_: wv_sb[:, d_, :]

        prj = pa.tile([P, 512], _F32, tag="prj", bufs=5,
                      name=f"prj_{lc}_{et}")
        for dti in range(NDT):
            nc.tensor.matmul(prj[:], w_sl(dti), xT[:, dti, ls],
                             start=(dti == 0), stop=(dti == NDT - 1))
        qs = wsb.tile([P, 512], _BF, tag="qs", bufs=3, name=f"qs_{lc}_{et}")
        nc.vector.tensor_copy(qs[:], prj[:])
        if et == 5:
            vtp = pa.tile([P, 512], _BF, tag="vtp", bufs=1,
                          name=f"vtp_{lc}")
            for j in range(4):
                nc.tensor.matmul(vtp[:, j * P:(j + 1) * P],
                                 qs[:, j * P:(j + 1) * P], id_sb[:],
                                 is_transpose=True, skip_group_check=True)
            nc.vector.tensor_copy(vn[:, lc * 4:lc * 4 + 4, :],
                                  vtp[:].rearrange("p (a b) -> p a b", a=4))
        else:
            qrot = pa.tile([P, 512], _F32, tag="qrot", bufs=2,
                           name=f"qrot_{lc}_{et}")
            nc.tensor.matmul(qrot[:], perm_sb[:], qs[:], start=True,
                             stop=True)
            tt = wsb.tile([P, 512], _BF, tag="tt", bufs=2,
                          name=f"tt_{lc}_{et}")
            nc.vector.tensor_mul(tt[:], qs[:], cos_sb[:, ls])
            uu = wsb.tile([P, 512], _BF, tag="uu", bufs=2,
                          name=f"uu_{lc}_{et}")
            nc.vector.tensor_mul(uu[:], qrot[:], sin_sb[:, ls])
            dest = qT[:, et, ls] if et < 4 else kT[:, ls]
            nc.vector.tensor_add(dest, tt[:], uu[:])


def _op_group(nc, pool, wsb, lt, mc, oT, wo_sb, y_d, bufs):
    """One output-projection PSUM group: 4 head-matmuls -> DVE copy -> DMA."""
    py = pool.tile([P, 512], _F32, tag="py", bufs=bufs,
                   name=f"py_{lt}_{mc}")
    for h in range(NHL):
        nc.tensor.matmul(py[:], oT[:, h, lt * P:(lt + 1) * P],
                         wo_sb[:, h, mc * 512:(mc + 1) * 512],
                         start=(h == 0), stop=(h == NHL - 1))
    ysb = wsb.tile([P, 512], _F32, tag="ysb", bufs=4, name=f"ysb_{lt}_{mc}")
    nc.vector.tensor_copy(ysb[:], py[:])
    nc.sync.dma_start(y_d[lt * P:(lt + 1) * P, mc * 512:(mc + 1) * 512],
                      ysb[:])


def _attn_chunk(nc, tc, pb, wsb, qi, qT, kT, vn, oT, ones_sb, msk_sb,
                id_sb, op_iter, op_args):
    """Causal attention for one 512-query chunk, all 4 heads.

    Two heads run in a software pipeline over key tiles so the PE never
    waits on the ACT exp of the tile it is about to consume.  When
    ``op_iter`` is set, one output-projection group of the previous
    chunk is interleaved per round as additional exp-latency cover
    (lookahead drops to 1 to fit PSUM: sc3+po2+ps2+py1 banks).
    """
    q0 = qi * 512
    nvis = 4 * qi
    nkt = nvis + 4
    look = 2 if op_iter is None else 1
    sc_bufs = 4 if op_iter is None else 2
    ops = list(op_iter) if op_iter is not None else []

    def op_left():
        return len(ops)

    def emit_op(bufs=2):
        if ops:
            lt, mc = ops.pop(0)
            _op_group(nc, pb, wsb, lt, mc, *op_args, bufs=bufs)

    for pair in ((0, 1), (2, 3)):
        # po/ps tiles are allocated lazily (at the first consumer round)
        # so the sc/py tags claim the low PSUM banks: the next segment's
        # pool then reuses early-freed banks first instead of WAR-waiting
        # on the pair-end reciprocal/normalization reads of po/ps.
        po = {}
        ps = {}
        es = {}

        def get_po_ps(h):
            if h not in po:
                po[h] = pb.tile([P, 512], _F32, tag="po", bufs=2,
                                name=f"po_{qi}_{h}")
                ps[h] = pb.tile([P, 512], _F32, tag="ps", bufs=2,
                                name=f"ps_{qi}_{h}")
            return po[h], ps[h]

        def emit_s(h, kt):
            off = max(0, (kt - nvis) * P)
            cs = slice(off, 512)
            sc = pb.tile([P, 512], _F32, tag="sc", bufs=sc_bufs,
                         name=f"sc_{qi}_{h}_{kt}")
            ktile = kT[:, kt * P:(kt + 1) * P]
            qtile = lambda o: qT[:, h, q0 + o:q0 + 512]
            if kt >= nvis:
                # diagonal tile: mask const first, S accumulates on top
                nc.tensor.matmul(sc[:, off:off + P], id_sb[:], msk_sb[:],
                                 start=True, stop=False,
                                 skip_group_check=True)
                nc.tensor.matmul(sc[:, off:off + P], ktile,
                                 qT[:, h, q0 + off:q0 + off + P],
                                 start=False, stop=True,
                                 skip_group_check=True)
                if off + P < 512:
                    nc.tensor.matmul(sc[:, off + P:512], ktile,
                                     qtile(off + P), start=True, stop=True,
                                     skip_group_check=True)
            else:
                nc.tensor.matmul(sc[:, cs], ktile, qtile(off),
                                 start=True, stop=True,
                                 skip_group_check=True)
            e = wsb.tile([P, 512], _BF, tag="es", bufs=8,
                         name=f"es_{qi}_{h}_{kt}")
            nc.scalar.activation(e[:, cs], sc[:, cs], _EXP, scale=SM_SCALE)
            es[(h, kt)] = e

        def emit_c(h, kt):
            off = max(0, (kt - nvis) * P)
            cs = slice(off, 512)
            e = es.pop((h, kt))
            poh, psh = get_po_ps(h)
            nc.tensor.matmul(psh[:, cs], ones_sb[:], e[:, cs],
                             start=(kt == 0), stop=(kt == nkt - 1),
                             skip_group_check=True)
            nc.tensor.matmul(poh[:, cs], vn[:, kt, :], e[:, cs],
                             start=(kt == 0), stop=(kt == nkt - 1),
                             skip_group_check=True)

        for kt in range(nkt + look):
            if kt < nkt:
                for h in pair:
                    emit_s(h, kt)
            if op_left() > 3:
                emit_op()
            if kt >= look:
                for h in pair:
                    emit_c(h, kt - look)

        for h in pair:
            rec = wsb.tile([P, 512], _F32, tag="rec", bufs=2,
                           name=f"rec_{qi}_{h}")
            nc.vector.reciprocal(rec[:], ps[h][:])
            nc.vector.tensor_mul(oT[:, h, q0:q0 + 512], po[h][:], rec[:])
        # emit held-back OP groups under the pair-drain / normalization
        # window so the PE stays busy while DVE reads down po/ps
        while op_left() > (3 if pair[0] == 0 else 0):
            emit_op()


def _body(nc, tc, xT_d, wq_d, wk_d, wv_d, wo_d, cos_d, sin_d,
          perm_d, ones_d, msk_d, id_d, y_d):
    from contextlib import ExitStack
    ctx = ExitStack()
    with ctx:
        pp = ctx.enter_context(tc.tile_pool(name="persist", bufs=1))
        wsb = ctx.enter_context(tc.tile_pool(name="wsb", bufs=2))

        xT = pp.tile([P, NDT, L], _BF, tag="xT")
        wq_sb = pp.tile([P, NHL, NDT, 128], _BF, tag="wq")
        wk_sb = pp.tile([P, NDT, 128], _BF, tag="wk")
        wv_sb = pp.tile([P, NDT, 128], _BF, tag="wv")
        wo_sb = pp.tile([P, NHL, L], _BF, tag="wo")
        cos_sb = pp.tile([P, L], _BF, tag="cos")
        sin_sb = pp.tile([P, L], _BF, tag="sin")
        perm_sb = pp.tile([P, P], _BF, tag="perm")
        ones_sb = pp.tile([P, P], _BF, tag="ones")
        msk_sb = pp.tile([P, P], _BF, tag="msk")
        id_sb = pp.tile([P, P], _BF, tag="ident")
        qT = pp.tile([P, NHL, L], _BF, tag="qT")
        kT = pp.tile([P, L], _BF, tag="kT")
        vn = pp.tile([P, NKT, 128], _BF, tag="vn")
        oT = pp.tile([P, NHL, L], _BF, tag="oT")

        # DMA issue order tracks compute order (sync-engine DMAs are
        # FIFO and hold the SP sequencer while waiting on data).
        nc.sync.dma_start(wk_sb[:], wk_d[:])
        nc.sync.dma_start(xT[:, 0:4, 0:512], xT_d[:, 0:4, 0:512])
        nc.sync.dma_start(xT[:, 4:8, 0:512], xT_d[:, 4:8, 0:512])
        nc.sync.dma_start(xT[:, 8:16, 0:512], xT_d[:, 8:16, 0:512])
        nc.sync.dma_start(wv_sb[:], wv_d[:])
        nc.sync.dma_start(id_sb[:], id_d[:])
        nc.sync.dma_start(perm_sb[:], perm_d[:])
        nc.sync.dma_start(cos_sb[:], cos_d[:])
        nc.sync.dma_start(sin_sb[:], sin_d[:])
        for hq in range(4):
            nc.sync.dma_start(wq_sb[:, hq], wq_d[:, hq])
        nc.sync.dma_start(ones_sb[:], ones_d[:])
        nc.sync.dma_start(msk_sb[:], msk_d[:])
        for lc in range(1, NLC):
            ls = slice(lc * 512, (lc + 1) * 512)
            nc.sync.dma_start(xT[:, :, ls], xT_d[:, :, ls])
        nc.sync.dma_start(wo_sb[:], wo_d[:])

        # interleaved projection / attention chunks; attention chunk qi
        # also consumes the output-projection groups of chunk qi-1
        op_args = (oT, wo_sb, y_d)
        for lc in range(NLC):
            with tc.tile_pool(name=f"pa{lc}", bufs=1, space="PSUM") as pa:
                _proj_segment(nc, tc, pa, wsb, lc, xT, wq_sb, wk_sb,
                              wv_sb, cos_sb, sin_sb, perm_sb, id_sb,
                              qT, kT, vn)
            op_iter = None
            if lc > 0:
                op_iter = iter([(lt, mc) for lt in range(4 * (lc - 1),
                                                        4 * lc)
                                for mc in range(4)])
            with tc.tile_pool(name=f"pb{lc}", bufs=1, space="PSUM") as pb:
                _attn_chunk(nc, tc, pb, wsb, lc, qT, kT, vn, oT,
                            ones_sb, msk_sb, id_sb, op_iter, op_args)

        # last chunk's output projection, streamed out per 128x512 tile;
        # the final group copies/DMAs in 128-col slivers to cut the
        # end-of-kernel matmul->copy->DMA drain latency
        with tc.tile_pool(name="pc", bufs=1, space="PSUM") as pc:
            for lt in range(12, 16):
                for mc in range(4):
                    if lt == 15 and mc == 3:
                        py = pc.tile([P, 512], _F32, tag="py", bufs=4,
                                     name="py_last")
                        for h in range(NHL):
                            nc.tensor.matmul(
                                py[:], oT[:, h, lt * P:(lt + 1) * P],
                                wo_sb[:, h, mc * 512:(mc + 1) * 512],
                                start=(h == 0), stop=(h == NHL - 1))
                        for sv in range(4):
                            svs = slice(sv * 128, (sv + 1) * 128)
                            ysb = wsb.tile([P, 128], _F32, tag="ysl",
                                           bufs=4, name=f"ysl_{sv}")
                            nc.vector.tensor_copy(ysb[:], py[:, svs])
                            nc.sync.dma_start(
                                y_d[lt * P:(lt + 1) * P,
                                    mc * 512 + sv * 128:
                                    mc * 512 + (sv + 1) * 128],
                                ysb[:])
                    else:
                        _op_group(nc, pc, wsb, lt, mc, oT, wo_sb, y_d,
                                  bufs=4)


def host_constants():
    inv = (1.0 / (10000.0 ** (np.arange(0, HD, 2, dtype=np.float32) / HD))
           ).astype(np.float32)
    t = np.arange(L, dtype=np.float32)
    freqs = t[:, None] * inv[None, :]                    # [L, 64]
    emb = np.concatenate([freqs, freqs], axis=-1)        # [L, 128]
    cosT = np.ascontiguousarray(np.cos(emb).T).astype(BF16)
    sinT = np.ascontiguousarray(np.sin(emb).T).astype(BF16)
    perm = np.zeros((P, P), dtype=F32)
    for i in range(64):
        perm[i + 64, i] = -1.0      # qrot[d] = -q[d+64],  d < 64
        perm[i, i + 64] = 1.0       # qrot[d] =  q[d-64],  d >= 64
    ones = np.ones((P, P), dtype=F32)
    # msk[k, t] = MASK_NEG where key k > query t (strict upper part per
    # diagonal 128-block); added into PSUM before the S matmul.
    msk = np.where(np.arange(P)[:, None] > np.arange(P)[None, :],
                   MASK_NEG, 0.0).astype(F32)
    ident = np.eye(P, dtype=F32)
    return {
        "cosT": cosT, "sinT": sinT,
        "perm": perm.astype(BF16), "ones": ones.astype(BF16),
        "msk": msk.astype(BF16), "ident": ident.astype(BF16),
    }


def make_in_map(consts, x, Wq, Wk, Wv, Wo, b, g):
    qs = slice(g * 512, (g + 1) * 512)
    kvs = slice(g * 128, (g + 1) * 128)
    wq = np.ascontiguousarray(
        Wq[qs].T.reshape(NDT, P, NHL, 128).transpose(1, 2, 0, 3)
    ).astype(BF16)
    wk = np.ascontiguousarray(
        Wk[kvs].T.reshape(NDT, P, 128).transpose(1, 0, 2)).astype(BF16)
    wv = np.ascontiguousarray(
        Wv[kvs].T.reshape(NDT, P, 128).transpose(1, 0, 2)).astype(BF16)
    wo = np.ascontiguousarray(
        Wo[:, qs].T.reshape(NHL, P, D).transpose(1, 0, 2)).astype(BF16)
    xT = np.ascontiguousarray(
        x[b].T.reshape(NDT, P, L).transpose(1, 0, 2)).astype(BF16)
    return {
        "xT": xT,
        "wq": wq, "wk": wk, "wv": wv, "wo": wo,
        **consts,
    }


_NC_CACHE = {}


def get_nc():
    if "nc" not in _NC_CACHE:
        _NC_CACHE["nc"] = build_nc()
    return _NC_CACHE["nc"]


def kernel(x, Wq, Wk, Wv, Wo):
    x = np.asarray(x, dtype=F32)
    Wq = np.asarray(Wq, dtype=F32)
    Wk = np.asarray(Wk, dtype=F32)
    Wv = np.asarray(Wv, dtype=F32)
    Wo = np.asarray(Wo, dtype=F32)
    nc = get_nc()
    consts = host_constants()
    in_maps = [make_in_map(consts, x, Wq, Wk, Wv, Wo, c // 4, c % 4)
               for c in range(8)]
    res = run_bass_kernel_spmd(nc, in_maps, list(range(8)))
    outs = [r["y"].astype(np.float64) for r in res.results]
    y = np.stack([sum(outs[0:4]), sum(outs[4:8])], axis=0).astype(F32)
    return y


# revision 21
# speedup vs baseline: 1.2386x; 1.0148x over previous
"""Causal GQA self-attention (RoPE) Trainium2 Bass kernel, 8-core SPMD.

Sharding: core c -> (b = c//4, g = c%4).  Data-parallel over batch B=2,
tensor-parallel over the 4 KV groups (4 query heads + 1 KV head each).
Each core computes a partial output y_bg = attn_out_g @ Wo[:, g-block].T
for its batch; the host sums the 4 group partials per batch (row-parallel
linear unshard).

v2 layout (all matmuls bf16, f32 PSUM accumulation):
  xT is transposed on the HOST and DMA'd as [128, 16, L] bf16 (no PE
  transposes).  DMA issue order matches compute order so the PE starts
  ~6us in.  Projection chunks (512 queries) and attention chunks are
  interleaved so the ACT engine's exp work overlaps projection matmuls.
  Attention runs a 2-head, lookahead-2 software pipeline per chunk:
  S-matmul groups run two key-tiles ahead of their exp consumers, which
  hides the ACT exp latency that dominated the v1 stalls.  The causal
  mask is applied with a PE matmul (identity x mask-const accumulated
  into PSUM before the S matmul) instead of a DVE multiply, keeping the
  softmax critical path PE->ACT->PE only.  Output projection runs last
  with 4 rotating PSUM banks; PSUM->SBUF copies run on the ACT engine
  and y tiles stream out per 128x512 block.
"""

import math
import sys

import numpy as np

try:
    import concourse.bass as bass  # noqa: F401
except ImportError:  # pragma: no cover
    sys.path.insert(0, "/opt/trn_rl_repo")
    import concourse.bass as bass  # noqa: F401

import ml_dtypes

import concourse.bacc as bacc
import concourse.mybir as mybir
import concourse.tile as tile
from concourse.bass_utils import run_bass_kernel_spmd

BF16 = ml_dtypes.bfloat16
F32 = np.float32

B, L, D = 2, 2048, 2048
HD = 128          # head dim
NHL = 4           # query heads per core (one KV group)
P = 128
NDT = D // P      # 16 d-tiles
NKT = L // P      # 16 key tiles
NLC = L // 512    # 4 512-wide l chunks
SM_SCALE = 1.0 / math.sqrt(HD)
MASK_NEG = -30000.0

_BF = mybir.dt.bfloat16
_F32 = mybir.dt.float32
_EXP = mybir.ActivationFunctionType.Exp
_COPY = mybir.ActivationFunctionType.Copy


def build_nc():
    nc = bacc.Bacc("TRN2", target_bir_lowering=False, debug=False,
                   enable_asserts=False)

    xT_d = nc.dram_tensor("xT", [P, NDT, L], _BF, kind="ExternalInput").ap()
    wq_d = nc.dram_tensor("wq", [P, NHL, NDT, 128], _BF,
                          kind="ExternalInput").ap()
    wk_d = nc.dram_tensor("wk", [P, NDT, 128], _BF, kind="ExternalInput").ap()
    wv_d = nc.dram_tensor("wv", [P, NDT, 128], _BF, kind="ExternalInput").ap()
    wo_d = nc.dram_tensor("wo", [P, NHL, L], _BF, kind="ExternalInput").ap()
    cos_d = nc.dram_tensor("cosT", [P, L], _BF, kind="ExternalInput").ap()
    sin_d = nc.dram_tensor("sinT", [P, L], _BF, kind="ExternalInput").ap()
    perm_d = nc.dram_tensor("perm", [P, P], _BF, kind="ExternalInput").ap()
    ones_d = nc.dram_tensor("ones", [P, P], _BF, kind="ExternalInput").ap()
    msk_d = nc.dram_tensor("msk", [P, P], _BF, kind="ExternalInput").ap()
    id_d = nc.dram_tensor("ident", [P, P], _BF, kind="ExternalInput").ap()
    y_d = nc.dram_tensor("y", [L, D], _F32, kind="ExternalOutput").ap()

    with tile.TileContext(nc) as tc:
        _body(nc, tc, xT_d, wq_d, wk_d, wv_d, wo_d, cos_d, sin_d,
              perm_d, ones_d, msk_d, id_d, y_d)
    nc.compile()
    return nc


def _proj_segment(nc, tc, pa, wsb, lc, xT, wq_sb, wk_sb, wv_sb,
                  cos_sb, sin_sb, perm_sb, id_sb, qT, kT, vn):
    """Q/K/V projections + RoPE for one 512-query chunk."""
    ls = slice(lc * 512, (lc + 1) * 512)
    for et in (4, 5, 0, 1, 2, 3):          # k, v, then the 4 q heads
        if et < 4:
            w_sl = lambda d_: wq_sb[:, et, d_, :]
        elif et == 4:
            w_sl = lambda d_: wk_sb[:, d_, :]
        else:
            w_sl = lambda d_: wv_sb[:, d_, :]

        prj = pa.tile([P, 512], _F32, tag="prj", bufs=5,
                      name=f"prj_{lc}_{et}")
        for dti in range(NDT):
            nc.tensor.matmul(prj[:], w_sl(dti), xT[:, dti, ls],
                             start=(dti == 0), stop=(dti == NDT - 1))
        qs = wsb.tile([P, 512], _BF, tag="qs", bufs=3, name=f"qs_{lc}_{et}")
        nc.vector.tensor_copy(qs[:], prj[:])
        if et == 5:
            vtp = pa.tile([P, 512], _BF, tag="vtp", bufs=1,
                          name=f"vtp_{lc}")
            for j in range(4):
                nc.tensor.matmul(vtp[:, j * P:(j + 1) * P],
                                 qs[:, j * P:(j + 1) * P], id_sb[:],
                                 is_transpose=True, skip_group_check=True)
            nc.vector.tensor_copy(vn[:, lc * 4:lc * 4 + 4, :],
                                  vtp[:].rearrange("p (a b) -> p a b", a=4))
        else:
            qrot = pa.tile([P, 512], _F32, tag="qrot", bufs=2,
                           name=f"qrot_{lc}_{et}")
            nc.tensor.matmul(qrot[:], perm_sb[:], qs[:], start=True,
                             stop=True)
            tt = wsb.tile([P, 512], _BF, tag="tt", bufs=2,
                          name=f"tt_{lc}_{et}")
            nc.vector.tensor_mul(tt[:], qs[:], cos_sb[:, ls])
            uu = wsb.tile([P, 512], _BF, tag="uu", bufs=2,
                          name=f"uu_{lc}_{et}")
            nc.vector.tensor_mul(uu[:], qrot[:], sin_sb[:, ls])
            dest = qT[:, et, ls] if et < 4 else kT[:, ls]
            nc.vector.tensor_add(dest, tt[:], uu[:])


def _op_group(nc, pool, wsb, lt, mc, oT, wo_sb, y_d, bufs, on_act=False):
    """One output-projection PSUM group: 4 head-matmuls -> copy -> DMA.

    The PSUM->SBUF copy runs on DVE by default; ``on_act`` routes it to
    the ACT engine for windows where DVE is serialized on the softmax
    normalization chain (pair drains, final phase)."""
    py = pool.tile([P, 512], _F32, tag="py", bufs=bufs,
                   name=f"py_{lt}_{mc}")
    for h in range(NHL):
        nc.tensor.matmul(py[:], oT[:, h, lt * P:(lt + 1) * P],
                         wo_sb[:, h, mc * 512:(mc + 1) * 512],
                         start=(h == 0), stop=(h == NHL - 1))
    ysb = wsb.tile([P, 512], _F32, tag="ysb", bufs=4, name=f"ysb_{lt}_{mc}")
    if on_act:
        nc.scalar.activation(ysb[:], py[:], _COPY)
    else:
        nc.vector.tensor_copy(ysb[:], py[:])
    nc.sync.dma_start(y_d[lt * P:(lt + 1) * P, mc * 512:(mc + 1) * 512],
                      ysb[:])


def _attn_chunk(nc, tc, pb, wsb, qi, qT, kT, vn, oT, ones_sb, msk_sb,
                id_sb, op_iter, op_args):
    """Causal attention for one 512-query chunk, all 4 heads.

    Two heads run in a software pipeline over key tiles so the PE never
    waits on the ACT exp of the tile it is about to consume.  When
    ``op_iter`` is set, one output-projection group of the previous
    chunk is interleaved per round as additional exp-latency cover
    (lookahead drops to 1 to fit PSUM: sc3+po2+ps2+py1 banks).
    """
    q0 = qi * 512
    nvis = 4 * qi
    nkt = nvis + 4
    look = 2 if op_iter is None else 1
    sc_bufs = 4 if op_iter is None else 2
    ops = list(op_iter) if op_iter is not None else []

    def op_left():
        return len(ops)

    def emit_op(on_act=False):
        if ops:
            lt, mc = ops.pop(0)
            _op_group(nc, pb, wsb, lt, mc, *op_args, bufs=2, on_act=on_act)

    for pair in ((0, 1), (2, 3)):
        # po/ps tiles are allocated lazily (at the first consumer round)
        # so the sc/py tags claim the low PSUM banks: the next segment's
        # pool then reuses early-freed banks first instead of WAR-waiting
        # on the pair-end reciprocal/normalization reads of po/ps.
        po = {}
        ps = {}
        es = {}

        def get_po_ps(h):
            if h not in po:
                po[h] = pb.tile([P, 512], _F32, tag="po", bufs=2,
                                name=f"po_{qi}_{h}")
                ps[h] = pb.tile([P, 512], _F32, tag="ps", bufs=2,
                                name=f"ps_{qi}_{h}")
            return po[h], ps[h]

        def emit_s(h, kt):
            off = max(0, (kt - nvis) * P)
            cs = slice(off, 512)
            sc = pb.tile([P, 512], _F32, tag="sc", bufs=sc_bufs,
                         name=f"sc_{qi}_{h}_{kt}")
            ktile = kT[:, kt * P:(kt + 1) * P]
            qtile = lambda o: qT[:, h, q0 + o:q0 + 512]
            if kt >= nvis:
                # diagonal tile: mask const first, S accumulates on top
                nc.tensor.matmul(sc[:, off:off + P], id_sb[:], msk_sb[:],
                                 start=True, stop=False,
                                 skip_group_check=True)
                nc.tensor.matmul(sc[:, off:off + P], ktile,
                                 qT[:, h, q0 + off:q0 + off + P],
                                 start=False, stop=True,
                                 skip_group_check=True)
                if off + P < 512:
                    nc.tensor.matmul(sc[:, off + P:512], ktile,
                                     qtile(off + P), start=True, stop=True,
                                     skip_group_check=True)
            else:
                nc.tensor.matmul(sc[:, cs], ktile, qtile(off),
                                 start=True, stop=True,
                                 skip_group_check=True)
            e = wsb.tile([P, 512], _BF, tag="es", bufs=8,
                         name=f"es_{qi}_{h}_{kt}")
            nc.scalar.activation(e[:, cs], sc[:, cs], _EXP, scale=SM_SCALE)
            es[(h, kt)] = e

        def emit_c(h, kt):
            off = max(0, (kt - nvis) * P)
            cs = slice(off, 512)
            e = es.pop((h, kt))
            poh, psh = get_po_ps(h)
            nc.tensor.matmul(psh[:, cs], ones_sb[:], e[:, cs],
                             start=(kt == 0), stop=(kt == nkt - 1),
                             skip_group_check=True)
            nc.tensor.matmul(poh[:, cs], vn[:, kt, :], e[:, cs],
                             start=(kt == 0), stop=(kt == nkt - 1),
                             skip_group_check=True)

        reserve = 6 if pair[0] == 0 else 3
        for kt in range(nkt + look):
            if kt < nkt:
                for h in pair:
                    emit_s(h, kt)
            if op_left() > reserve:
                emit_op()
            if kt >= look:
                for h in pair:
                    emit_c(h, kt - look)

        for h in pair:
            rec = wsb.tile([P, 512], _F32, tag="rec", bufs=2,
                           name=f"rec_{qi}_{h}")
            nc.vector.reciprocal(rec[:], ps[h][:])
            nc.vector.tensor_mul(oT[:, h, q0:q0 + 512], po[h][:], rec[:])
        # emit held-back OP groups under the pair-drain / normalization
        # window so the PE stays busy while DVE reads down po/ps; their
        # copies run on ACT (idle here) so the pool's last reads retire
        # in parallel with the DVE chain
        while op_left() > (3 if pair[0] == 0 else 0):
            emit_op(on_act=True)


def _body(nc, tc, xT_d, wq_d, wk_d, wv_d, wo_d, cos_d, sin_d,
          perm_d, ones_d, msk_d, id_d, y_d):
    from contextlib import ExitStack
    ctx = ExitStack()
    with ctx:
        pp = ctx.enter_context(tc.tile_pool(name="persist", bufs=1))
        wsb = ctx.enter_context(tc.tile_pool(name="wsb", bufs=2))

        xT = pp.tile([P, NDT, L], _BF, tag="xT")
        wq_sb = pp.tile([P, NHL, NDT, 128], _BF, tag="wq")
        wk_sb = pp.tile([P, NDT, 128], _BF, tag="wk")
        wv_sb = pp.tile([P, NDT, 128], _BF, tag="wv")
        wo_sb = pp.tile([P, NHL, L], _BF, tag="wo")
        cos_sb = pp.tile([P, L], _BF, tag="cos")
        sin_sb = pp.tile([P, L], _BF, tag="sin")
        perm_sb = pp.tile([P, P], _BF, tag="perm")
        ones_sb = pp.tile([P, P], _BF, tag="ones")
        msk_sb = pp.tile([P, P], _BF, tag="msk")
        id_sb = pp.tile([P, P], _BF, tag="ident")
        qT = pp.tile([P, NHL, L], _BF, tag="qT")
        kT = pp.tile([P, L], _BF, tag="kT")
        vn = pp.tile([P, NKT, 128], _BF, tag="vn")
        oT = pp.tile([P, NHL, L], _BF, tag="oT")

        # DMA issue order tracks compute order (sync-engine DMAs are
        # FIFO and hold the SP sequencer while waiting on data).
        nc.sync.dma_start(wk_sb[:], wk_d[:])
        nc.sync.dma_start(xT[:, 0:4, 0:512], xT_d[:, 0:4, 0:512])
        nc.sync.dma_start(xT[:, 4:8, 0:512], xT_d[:, 4:8, 0:512])
        nc.sync.dma_start(xT[:, 8:16, 0:512], xT_d[:, 8:16, 0:512])
        nc.sync.dma_start(wv_sb[:], wv_d[:])
        nc.sync.dma_start(id_sb[:], id_d[:])
        nc.sync.dma_start(perm_sb[:], perm_d[:])
        nc.sync.dma_start(cos_sb[:], cos_d[:])
        nc.sync.dma_start(sin_sb[:], sin_d[:])
        for hq in range(4):
            nc.sync.dma_start(wq_sb[:, hq], wq_d[:, hq])
        nc.sync.dma_start(ones_sb[:], ones_d[:])
        nc.sync.dma_start(msk_sb[:], msk_d[:])
        for lc in range(1, NLC):
            ls = slice(lc * 512, (lc + 1) * 512)
            nc.sync.dma_start(xT[:, :, ls], xT_d[:, :, ls])
        nc.sync.dma_start(wo_sb[:], wo_d[:])

        # interleaved projection / attention chunks; attention chunk qi
        # also consumes the output-projection groups of chunk qi-1
        op_args = (oT, wo_sb, y_d)
        for lc in range(NLC):
            with tc.tile_pool(name=f"pa{lc}", bufs=1, space="PSUM") as pa:
                _proj_segment(nc, tc, pa, wsb, lc, xT, wq_sb, wk_sb,
                              wv_sb, cos_sb, sin_sb, perm_sb, id_sb,
                              qT, kT, vn)
            op_iter = None
            if lc > 0:
                op_iter = iter([(lt, mc) for lt in range(4 * (lc - 1),
                                                        4 * lc)
                                for mc in range(4)])
            with tc.tile_pool(name=f"pb{lc}", bufs=1, space="PSUM") as pb:
                _attn_chunk(nc, tc, pb, wsb, lc, qT, kT, vn, oT,
                            ones_sb, msk_sb, id_sb, op_iter, op_args)

        # last chunk's output projection, streamed out per 128x512 tile;
        # copies on ACT (idle in this phase, and DVE still owes the last
        # chunk's normalization chain)
        with tc.tile_pool(name="pc", bufs=1, space="PSUM") as pc:
            for lt in range(12, 16):
                for mc in range(4):
                    _op_group(nc, pc, wsb, lt, mc, oT, wo_sb, y_d,
                              bufs=4, on_act=True)


def host_constants():
    inv = (1.0 / (10000.0 ** (np.arange(0, HD, 2, dtype=np.float32) / HD))
           ).astype(np.float32)
    t = np.arange(L, dtype=np.float32)
    freqs = t[:, None] * inv[None, :]                    # [L, 64]
    emb = np.concatenate([freqs, freqs], axis=-1)        # [L, 128]
    cosT = np.ascontiguousarray(np.cos(emb).T).astype(BF16)
    sinT = np.ascontiguousarray(np.sin(emb).T).astype(BF16)
    perm = np.zeros((P, P), dtype=F32)
    for i in range(64):
        perm[i + 64, i] = -1.0      # qrot[d] = -q[d+64],  d < 64
        perm[i, i + 64] = 1.0       # qrot[d] =  q[d-64],  d >= 64
    ones = np.ones((P, P), dtype=F32)
    # msk[k, t] = MASK_NEG where key k > query t (strict upper part per
    # diagonal 128-block); added into PSUM before the S matmul.
    msk = np.where(np.arange(P)[:, None] > np.arange(P)[None, :],
                   MASK_NEG, 0.0).astype(F32)
    ident = np.eye(P, dtype=F32)
    return {
        "cosT": cosT, "sinT": sinT,
        "perm": perm.astype(BF16), "ones": ones.astype(BF16),
        "msk": msk.astype(BF16), "ident": ident.astype(BF16),
    }


def make_in_map(consts, x, Wq, Wk, Wv, Wo, b, g):
    qs = slice(g * 512, (g + 1) * 512)
    kvs = slice(g * 128, (g + 1) * 128)
    wq = np.ascontiguousarray(
        Wq[qs].T.reshape(NDT, P, NHL, 128).transpose(1, 2, 0, 3)
    ).astype(BF16)
    wk = np.ascontiguousarray(
        Wk[kvs].T.reshape(NDT, P, 128).transpose(1, 0, 2)).astype(BF16)
    wv = np.ascontiguousarray(
        Wv[kvs].T.reshape(NDT, P, 128).transpose(1, 0, 2)).astype(BF16)
    wo = np.ascontiguousarray(
        Wo[:, qs].T.reshape(NHL, P, D).transpose(1, 0, 2)).astype(BF16)
    xT = np.ascontiguousarray(
        x[b].T.reshape(NDT, P, L).transpose(1, 0, 2)).astype(BF16)
    return {
        "xT": xT,
        "wq": wq, "wk": wk, "wv": wv, "wo": wo,
        **consts,
    }


_NC_CACHE = {}


def get_nc():
    if "nc" not in _NC_CACHE:
        _NC_CACHE["nc"] = build_nc()
    return _NC_CACHE["nc"]


def kernel(x, Wq, Wk, Wv, Wo):
    x = np.asarray(x, dtype=F32)
    Wq = np.asarray(Wq, dtype=F32)
    Wk = np.asarray(Wk, dtype=F32)
    Wv = np.asarray(Wv, dtype=F32)
    Wo = np.asarray(Wo, dtype=F32)
    nc = get_nc()
    consts = host_constants()
    in_maps = [make_in_map(consts, x, Wq, Wk, Wv, Wo, c // 4, c % 4)
               for c in range(8)]
    res = run_bass_kernel_spmd(nc, in_maps, list(range(8)))
    outs = [r["y"].astype(np.float64) for r in res.results]
    y = np.stack([sum(outs[0:4]), sum(outs[4:8])], axis=0).astype(F32)
    return y


# revision 30
# speedup vs baseline: 1.2585x; 1.0160x over previous
"""Causal GQA self-attention (RoPE) Trainium2 Bass kernel, 8-core SPMD.

Sharding: core c -> (b = c//4, g = c%4).  Data-parallel over batch B=2,
tensor-parallel over the 4 KV groups (4 query heads + 1 KV head each).
Each core computes a partial output y_bg = attn_out_g @ Wo[:, g-block].T
for its batch; the host sums the 4 group partials per batch (row-parallel
linear unshard).

v2 layout (all matmuls bf16, f32 PSUM accumulation):
  xT is transposed on the HOST and DMA'd as [128, 16, L] bf16 (no PE
  transposes).  DMA issue order matches compute order so the PE starts
  ~6us in.  Projection chunks (512 queries) and attention chunks are
  interleaved so the ACT engine's exp work overlaps projection matmuls.
  Attention runs a 2-head, lookahead-2 software pipeline per chunk:
  S-matmul groups run two key-tiles ahead of their exp consumers, which
  hides the ACT exp latency that dominated the v1 stalls.  The causal
  mask is applied with a PE matmul (identity x mask-const accumulated
  into PSUM before the S matmul) instead of a DVE multiply, keeping the
  softmax critical path PE->ACT->PE only.  Output projection runs last
  with 4 rotating PSUM banks; PSUM->SBUF copies run on the ACT engine
  and y tiles stream out per 128x512 block.
"""

import math
import sys

import numpy as np

try:
    import concourse.bass as bass  # noqa: F401
except ImportError:  # pragma: no cover
    sys.path.insert(0, "/opt/trn_rl_repo")
    import concourse.bass as bass  # noqa: F401

import ml_dtypes

import concourse.bacc as bacc
import concourse.mybir as mybir
import concourse.tile as tile
from concourse.bass_utils import run_bass_kernel_spmd

BF16 = ml_dtypes.bfloat16
F32 = np.float32

B, L, D = 2, 2048, 2048
HD = 128          # head dim
NHL = 4           # query heads per core (one KV group)
P = 128
NDT = D // P      # 16 d-tiles
NKT = L // P      # 16 key tiles
NLC = L // 512    # 4 512-wide l chunks
SM_SCALE = 1.0 / math.sqrt(HD)
MASK_NEG = -30000.0

_BF = mybir.dt.bfloat16
_F32 = mybir.dt.float32
_EXP = mybir.ActivationFunctionType.Exp
_COPY = mybir.ActivationFunctionType.Copy


def build_nc():
    nc = bacc.Bacc("TRN2", target_bir_lowering=False, debug=False,
                   enable_asserts=False)

    xT_d = nc.dram_tensor("xT", [P, NDT, L], _BF, kind="ExternalInput").ap()
    wq_d = nc.dram_tensor("wq", [P, NHL, NDT, 128], _BF,
                          kind="ExternalInput").ap()
    wk_d = nc.dram_tensor("wk", [P, NDT, 128], _BF, kind="ExternalInput").ap()
    wv_d = nc.dram_tensor("wv", [P, NDT, 128], _BF, kind="ExternalInput").ap()
    wo_d = nc.dram_tensor("wo", [P, NHL, L], _BF, kind="ExternalInput").ap()
    cos_d = nc.dram_tensor("cosT", [P, L], _BF, kind="ExternalInput").ap()
    sin_d = nc.dram_tensor("sinT", [P, L], _BF, kind="ExternalInput").ap()
    perm_d = nc.dram_tensor("perm", [P, P], _BF, kind="ExternalInput").ap()
    ones_d = nc.dram_tensor("ones", [P, P], _BF, kind="ExternalInput").ap()
    msk_d = nc.dram_tensor("msk", [P, P], _BF, kind="ExternalInput").ap()
    id_d = nc.dram_tensor("ident", [P, P], _BF, kind="ExternalInput").ap()
    y_d = nc.dram_tensor("y", [L, D], _F32, kind="ExternalOutput").ap()

    with tile.TileContext(nc) as tc:
        _body(nc, tc, xT_d, wq_d, wk_d, wv_d, wo_d, cos_d, sin_d,
              perm_d, ones_d, msk_d, id_d, y_d)
    nc.compile()
    return nc


def _proj_segment(nc, tc, pa, wsb, lc, xT, wq_sb, wk_sb, wv_sb,
                  cos_sb, sin_sb, perm_sb, id_sb, qT, kT, vn,
                  deferred=None):
    """Q/K/V projections + RoPE for one 512-query chunk.

    Uses only PSUM banks 0-3 (prj x2, qrot, vtp) so the previous
    attention chunk's po/ps banks (4-7) stay untouched: its deferred
    normalization chain (``deferred``) is emitted after the first
    projection group and overlaps this segment's matmuls instead of
    stalling the pool-open."""
    ls = slice(lc * 512, (lc + 1) * 512)
    for ei, et in enumerate((4, 5, 0, 1, 2, 3)):   # k, v, then 4 q heads
        if et < 4:
            w_sl = lambda d_: wq_sb[:, et, d_, :]
        elif et == 4:
            w_sl = lambda d_: wk_sb[:, d_, :]
        else:
            w_sl = lambda d_: wv_sb[:, d_, :]

        if ei == 1 and deferred:
            for fn in deferred:
                fn()
            deferred = None
        prj = pa.tile([P, 512], _F32, tag="prj", bufs=2,
                      name=f"prj_{lc}_{et}")
        for dti in range(NDT):
            nc.tensor.matmul(prj[:], w_sl(dti), xT[:, dti, ls],
                             start=(dti == 0), stop=(dti == NDT - 1))
        qs = wsb.tile([P, 512], _BF, tag="qs", bufs=3, name=f"qs_{lc}_{et}")
        nc.vector.tensor_copy(qs[:], prj[:])
        if et == 5:
            vtp = pa.tile([P, 512], _BF, tag="vtp", bufs=1,
                          name=f"vtp_{lc}")
            for j in range(4):
                nc.tensor.matmul(vtp[:, j * P:(j + 1) * P],
                                 qs[:, j * P:(j + 1) * P], id_sb[:],
                                 is_transpose=True, skip_group_check=True)
            nc.vector.tensor_copy(vn[:, lc * 4:lc * 4 + 4, :],
                                  vtp[:].rearrange("p (a b) -> p a b", a=4))
        else:
            qrot = pa.tile([P, 512], _F32, tag="qrot", bufs=1,
                           name=f"qrot_{lc}_{et}")
            nc.tensor.matmul(qrot[:], perm_sb[:], qs[:], start=True,
                             stop=True)
            tt = wsb.tile([P, 512], _BF, tag="tt", bufs=2,
                          name=f"tt_{lc}_{et}")
            nc.vector.tensor_mul(tt[:], qs[:], cos_sb[:, ls])
            uu = wsb.tile([P, 512], _BF, tag="uu", bufs=2,
                          name=f"uu_{lc}_{et}")
            nc.vector.tensor_mul(uu[:], qrot[:], sin_sb[:, ls])
            dest = qT[:, et, ls] if et < 4 else kT[:, ls]
            nc.vector.tensor_add(dest, tt[:], uu[:])


def _op_group(nc, pool, wsb, lt, mc, oT, wo_sb, y_d, bufs, on_act=False):
    """One output-projection PSUM group: 4 head-matmuls -> copy -> DMA.

    The PSUM->SBUF copy runs on DVE by default; ``on_act`` routes it to
    the ACT engine for windows where DVE is serialized on the softmax
    normalization chain (pair drains, final phase)."""
    py = pool.tile([P, 512], _F32, tag="py", bufs=bufs,
                   name=f"py_{lt}_{mc}")
    for h in range(NHL):
        nc.tensor.matmul(py[:], oT[:, h, lt * P:(lt + 1) * P],
                         wo_sb[:, h, mc * 512:(mc + 1) * 512],
                         start=(h == 0), stop=(h == NHL - 1))
    ysb = wsb.tile([P, 512], _F32, tag="ysb", bufs=4, name=f"ysb_{lt}_{mc}")
    if on_act:
        nc.scalar.activation(ysb[:], py[:], _COPY)
    else:
        nc.vector.tensor_copy(ysb[:], py[:])
    nc.sync.dma_start(y_d[lt * P:(lt + 1) * P, mc * 512:(mc + 1) * 512],
                      ysb[:])


def _attn_chunk(nc, tc, pb, wsb, qi, qT, kT, vn, oT, ones_sb, msk_sb,
                id_sb, op_iter, op_args):
    """Causal attention for one 512-query chunk, all 4 heads.

    Two heads run in a software pipeline over key tiles so the PE never
    waits on the ACT exp of the tile it is about to consume.  When
    ``op_iter`` is set, one output-projection group of the previous
    chunk is interleaved per round as additional exp-latency cover
    (lookahead drops to 1 to fit PSUM: sc3+po2+ps2+py1 banks).
    """
    q0 = qi * 512
    nvis = 4 * qi
    nkt = nvis + 4
    look = 2 if op_iter is None else 1
    sc_bufs = 4 if op_iter is None else 2
    ops = list(op_iter) if op_iter is not None else []
    deferred = []

    def op_left():
        return len(ops)

    def emit_op(on_act=True):
        if ops:
            lt, mc = ops.pop(0)
            _op_group(nc, pb, wsb, lt, mc, *op_args, bufs=2, on_act=on_act)

    for pair in ((0, 1), (2, 3)):
        # po/ps tiles are allocated lazily (at the first consumer round)
        # so the sc/py tags claim the low PSUM banks: the next segment's
        # pool then reuses early-freed banks first instead of WAR-waiting
        # on the pair-end reciprocal/normalization reads of po/ps.
        po = {}
        ps = {}
        es = {}

        def get_po_ps(h):
            if h not in po:
                po[h] = pb.tile([P, 512], _F32, tag="po", bufs=2,
                                name=f"po_{qi}_{h}")
                ps[h] = pb.tile([P, 512], _F32, tag="ps", bufs=2,
                                name=f"ps_{qi}_{h}")
            return po[h], ps[h]

        def emit_s(h, kt):
            off = max(0, (kt - nvis) * P)
            cs = slice(off, 512)
            sc = pb.tile([P, 512], _F32, tag="sc", bufs=sc_bufs,
                         name=f"sc_{qi}_{h}_{kt}")
            ktile = kT[:, kt * P:(kt + 1) * P]
            qtile = lambda o: qT[:, h, q0 + o:q0 + 512]
            if kt >= nvis:
                # diagonal tile: mask const first, S accumulates on top
                nc.tensor.matmul(sc[:, off:off + P], id_sb[:], msk_sb[:],
                                 start=True, stop=False,
                                 skip_group_check=True)
                nc.tensor.matmul(sc[:, off:off + P], ktile,
                                 qT[:, h, q0 + off:q0 + off + P],
                                 start=False, stop=True,
                                 skip_group_check=True)
                if off + P < 512:
                    nc.tensor.matmul(sc[:, off + P:512], ktile,
                                     qtile(off + P), start=True, stop=True,
                                     skip_group_check=True)
            else:
                nc.tensor.matmul(sc[:, cs], ktile, qtile(off),
                                 start=True, stop=True,
                                 skip_group_check=True)
            e = wsb.tile([P, 512], _BF, tag="es", bufs=8,
                         name=f"es_{qi}_{h}_{kt}")
            nc.scalar.activation(e[:, cs], sc[:, cs], _EXP, scale=SM_SCALE)
            es[(h, kt)] = e

        def emit_c(h, kt):
            off = max(0, (kt - nvis) * P)
            cs = slice(off, 512)
            e = es.pop((h, kt))
            poh, psh = get_po_ps(h)
            nc.tensor.matmul(psh[:, cs], ones_sb[:], e[:, cs],
                             start=(kt == 0), stop=(kt == nkt - 1),
                             skip_group_check=True)
            nc.tensor.matmul(poh[:, cs], vn[:, kt, :], e[:, cs],
                             start=(kt == 0), stop=(kt == nkt - 1),
                             skip_group_check=True)

        reserve = 6 if pair[0] == 0 else 3
        for kt in range(nkt + look):
            if kt < nkt:
                for h in pair:
                    emit_s(h, kt)
            if op_left() > reserve:
                emit_op()
            if kt >= look:
                for h in pair:
                    emit_c(h, kt - look)

        def normalize(h, poh, psh):
            def fn():
                rec = wsb.tile([P, 512], _F32, tag="rec", bufs=2,
                               name=f"rec_{qi}_{h}")
                nc.vector.reciprocal(rec[:], psh[:])
                nc.vector.tensor_mul(oT[:, h, q0:q0 + 512], poh[:], rec[:])
            return fn

        if pair[0] == 0 or qi == NLC - 1:
            for h in pair:
                normalize(h, po[h], ps[h])()
        else:
            # pair 2's normalization chain is deferred into the next
            # segment (which only touches PSUM banks 0-3) so the pool
            # boundary doesn't stall on it
            for h in pair:
                deferred.append(normalize(h, po[h], ps[h]))
        # emit held-back OP groups under the pair-drain window so the
        # PE stays busy while DVE/ACT drain the pool's last reads
        while op_left() > (3 if pair[0] == 0 else 0):
            emit_op()

    return deferred


def _body(nc, tc, xT_d, wq_d, wk_d, wv_d, wo_d, cos_d, sin_d,
          perm_d, ones_d, msk_d, id_d, y_d):
    from contextlib import ExitStack
    ctx = ExitStack()
    with ctx:
        pp = ctx.enter_context(tc.tile_pool(name="persist", bufs=1))
        wsb = ctx.enter_context(tc.tile_pool(name="wsb", bufs=2))

        xT = pp.tile([P, NDT, L], _BF, tag="xT")
        wq_sb = pp.tile([P, NHL, NDT, 128], _BF, tag="wq")
        wk_sb = pp.tile([P, NDT, 128], _BF, tag="wk")
        wv_sb = pp.tile([P, NDT, 128], _BF, tag="wv")
        wo_sb = pp.tile([P, NHL, L], _BF, tag="wo")
        cos_sb = pp.tile([P, L], _BF, tag="cos")
        sin_sb = pp.tile([P, L], _BF, tag="sin")
        perm_sb = pp.tile([P, P], _BF, tag="perm")
        ones_sb = pp.tile([P, P], _BF, tag="ones")
        msk_sb = pp.tile([P, P], _BF, tag="msk")
        id_sb = pp.tile([P, P], _BF, tag="ident")
        qT = pp.tile([P, NHL, L], _BF, tag="qT")
        kT = pp.tile([P, L], _BF, tag="kT")
        vn = pp.tile([P, NKT, 128], _BF, tag="vn")
        oT = pp.tile([P, NHL, L], _BF, tag="oT")

        # DMA issue order tracks compute order (sync-engine DMAs are
        # FIFO and hold the SP sequencer while waiting on data).
        nc.sync.dma_start(wk_sb[:], wk_d[:])
        nc.sync.dma_start(xT[:, 0:4, 0:512], xT_d[:, 0:4, 0:512])
        nc.sync.dma_start(xT[:, 4:8, 0:512], xT_d[:, 4:8, 0:512])
        nc.sync.dma_start(xT[:, 8:16, 0:512], xT_d[:, 8:16, 0:512])
        nc.sync.dma_start(wv_sb[:], wv_d[:])
        nc.sync.dma_start(id_sb[:], id_d[:])
        nc.sync.dma_start(perm_sb[:], perm_d[:])
        nc.sync.dma_start(cos_sb[:], cos_d[:])
        nc.sync.dma_start(sin_sb[:], sin_d[:])
        for hq in range(4):
            nc.sync.dma_start(wq_sb[:, hq], wq_d[:, hq])
        nc.sync.dma_start(ones_sb[:], ones_d[:])
        nc.sync.dma_start(msk_sb[:], msk_d[:])
        for lc in range(1, NLC):
            ls = slice(lc * 512, (lc + 1) * 512)
            nc.sync.dma_start(xT[:, :, ls], xT_d[:, :, ls])
        nc.sync.dma_start(wo_sb[:], wo_d[:])

        # interleaved projection / attention chunks; attention chunk qi
        # also consumes the output-projection groups of chunk qi-1
        op_args = (oT, wo_sb, y_d)
        deferred = []
        for lc in range(NLC):
            with tc.tile_pool(name=f"pa{lc}", bufs=1, space="PSUM") as pa:
                _proj_segment(nc, tc, pa, wsb, lc, xT, wq_sb, wk_sb,
                              wv_sb, cos_sb, sin_sb, perm_sb, id_sb,
                              qT, kT, vn, deferred=deferred)
            op_iter = None
            if lc > 0:
                op_iter = iter([(lt, mc) for lt in range(4 * (lc - 1),
                                                        4 * lc)
                                for mc in range(4)])
            with tc.tile_pool(name=f"pb{lc}", bufs=1, space="PSUM") as pb:
                deferred = _attn_chunk(nc, tc, pb, wsb, lc, qT, kT, vn,
                                       oT, ones_sb, msk_sb, id_sb,
                                       op_iter, op_args)

        # last chunk's output projection, streamed out per 128x512 tile;
        # copies alternate ACT/DVE so the end-of-kernel copy backlog
        # drains on two engines
        with tc.tile_pool(name="pc", bufs=1, space="PSUM") as pc:
            n = 0
            for lt in range(12, 16):
                for mc in range(4):
                    _op_group(nc, pc, wsb, lt, mc, oT, wo_sb, y_d,
                              bufs=4, on_act=(n % 2 == 0))
                    n += 1


def host_constants():
    inv = (1.0 / (10000.0 ** (np.arange(0, HD, 2, dtype=np.float32) / HD))
           ).astype(np.float32)
    t = np.arange(L, dtype=np.float32)
    freqs = t[:, None] * inv[None, :]                    # [L, 64]
    emb = np.concatenate([freqs, freqs], axis=-1)        # [L, 128]
    cosT = np.ascontiguousarray(np.cos(emb).T).astype(BF16)
    sinT = np.ascontiguousarray(np.sin(emb).T).astype(BF16)
    perm = np.zeros((P, P), dtype=F32)
    for i in range(64):
        perm[i + 64, i] = -1.0      # qrot[d] = -q[d+64],  d < 64
        perm[i, i + 64] = 1.0       # qrot[d] =  q[d-64],  d >= 64
    ones = np.ones((P, P), dtype=F32)
    # msk[k, t] = MASK_NEG where key k > query t (strict upper part per
    # diagonal 128-block); added into PSUM before the S matmul.
    msk = np.where(np.arange(P)[:, None] > np.arange(P)[None, :],
                   MASK_NEG, 0.0).astype(F32)
    ident = np.eye(P, dtype=F32)
    return {
        "cosT": cosT, "sinT": sinT,
        "perm": perm.astype(BF16), "ones": ones.astype(BF16),
        "msk": msk.astype(BF16), "ident": ident.astype(BF16),
    }


def make_in_map(consts, x, Wq, Wk, Wv, Wo, b, g):
    qs = slice(g * 512, (g + 1) * 512)
    kvs = slice(g * 128, (g + 1) * 128)
    wq = np.ascontiguousarray(
        Wq[qs].T.reshape(NDT, P, NHL, 128).transpose(1, 2, 0, 3)
    ).astype(BF16)
    wk = np.ascontiguousarray(
        Wk[kvs].T.reshape(NDT, P, 128).transpose(1, 0, 2)).astype(BF16)
    wv = np.ascontiguousarray(
        Wv[kvs].T.reshape(NDT, P, 128).transpose(1, 0, 2)).astype(BF16)
    wo = np.ascontiguousarray(
        Wo[:, qs].T.reshape(NHL, P, D).transpose(1, 0, 2)).astype(BF16)
    xT = np.ascontiguousarray(
        x[b].T.reshape(NDT, P, L).transpose(1, 0, 2)).astype(BF16)
    return {
        "xT": xT,
        "wq": wq, "wk": wk, "wv": wv, "wo": wo,
        **consts,
    }


_NC_CACHE = {}


def get_nc():
    if "nc" not in _NC_CACHE:
        _NC_CACHE["nc"] = build_nc()
    return _NC_CACHE["nc"]


def kernel(x, Wq, Wk, Wv, Wo):
    x = np.asarray(x, dtype=F32)
    Wq = np.asarray(Wq, dtype=F32)
    Wk = np.asarray(Wk, dtype=F32)
    Wv = np.asarray(Wv, dtype=F32)
    Wo = np.asarray(Wo, dtype=F32)
    nc = get_nc()
    consts = host_constants()
    in_maps = [make_in_map(consts, x, Wq, Wk, Wv, Wo, c // 4, c % 4)
               for c in range(8)]
    res = run_bass_kernel_spmd(nc, in_maps, list(range(8)))
    outs = [r["y"].astype(np.float64) for r in res.results]
    y = np.stack([sum(outs[0:4]), sum(outs[4:8])], axis=0).astype(F32)
    return y


# revision 42
# speedup vs baseline: 1.3010x; 1.0338x over previous
"""Causal GQA self-attention (RoPE) Trainium2 Bass kernel, 8-core SPMD.

Sharding: core c -> (b = c//4, g = c%4).  Data-parallel over batch B=2,
tensor-parallel over the 4 KV groups (4 query heads + 1 KV head each).
Each core computes a partial output y_bg = attn_out_g @ Wo[:, g-block].T
for its batch; the host sums the 4 group partials per batch (row-parallel
linear unshard).

v2 layout (all matmuls bf16, f32 PSUM accumulation):
  xT is transposed on the HOST and DMA'd as [128, 16, L] bf16 (no PE
  transposes).  DMA issue order matches compute order so the PE starts
  ~6us in.  Projection chunks (512 queries) and attention chunks are
  interleaved so the ACT engine's exp work overlaps projection matmuls.
  Attention runs a 2-head, lookahead-2 software pipeline per chunk:
  S-matmul groups run two key-tiles ahead of their exp consumers, which
  hides the ACT exp latency that dominated the v1 stalls.  The causal
  mask is applied with a PE matmul (identity x mask-const accumulated
  into PSUM before the S matmul) instead of a DVE multiply, keeping the
  softmax critical path PE->ACT->PE only.  Output projection runs last
  with 4 rotating PSUM banks; PSUM->SBUF copies run on the ACT engine
  and y tiles stream out per 128x512 block.
"""

import math
import sys

import numpy as np

try:
    import concourse.bass as bass  # noqa: F401
except ImportError:  # pragma: no cover
    sys.path.insert(0, "/opt/trn_rl_repo")
    import concourse.bass as bass  # noqa: F401

import ml_dtypes

import concourse.bacc as bacc
import concourse.bass_isa as bass_isa
import concourse.mybir as mybir
import concourse.tile as tile
from concourse.bass_utils import run_bass_kernel_spmd

BF16 = ml_dtypes.bfloat16
F32 = np.float32

B, L, D = 2, 2048, 2048
HD = 128          # head dim
NHL = 4           # query heads per core (one KV group)
P = 128
NDT = D // P      # 16 d-tiles
NKT = L // P      # 16 key tiles
NLC = L // 512    # 4 512-wide l chunks
SM_SCALE = 1.0 / math.sqrt(HD)
MASK_NEG = -30000.0

_BF = mybir.dt.bfloat16
_F32 = mybir.dt.float32
_EXP = mybir.ActivationFunctionType.Exp
_COPY = mybir.ActivationFunctionType.Copy


def build_nc():
    nc = bacc.Bacc("TRN2", target_bir_lowering=False, debug=False,
                   enable_asserts=False)

    xT_d = nc.dram_tensor("xT", [P, NDT, L], _BF, kind="ExternalInput").ap()
    wq_d = nc.dram_tensor("wq", [P, NHL, NDT, 128], _BF,
                          kind="ExternalInput").ap()
    wk_d = nc.dram_tensor("wk", [P, NDT, 128], _BF, kind="ExternalInput").ap()
    wv_d = nc.dram_tensor("wv", [P, NDT, 128], _BF, kind="ExternalInput").ap()
    wo_d = nc.dram_tensor("wo", [P, NHL, L], _BF, kind="ExternalInput").ap()
    cos_d = nc.dram_tensor("cosT", [P, L], _BF, kind="ExternalInput").ap()
    sin_d = nc.dram_tensor("sinT", [P, L], _BF, kind="ExternalInput").ap()
    perm_d = nc.dram_tensor("perm", [P, P], _BF, kind="ExternalInput").ap()
    ones_d = nc.dram_tensor("ones", [P, P], _BF, kind="ExternalInput").ap()
    msk_d = nc.dram_tensor("msk", [P, P], _BF, kind="ExternalInput").ap()
    id_d = nc.dram_tensor("ident", [P, P], _BF, kind="ExternalInput").ap()
    y_d = nc.dram_tensor("y", [L, D], _F32, kind="ExternalOutput").ap()

    with tile.TileContext(nc) as tc:
        _body(nc, tc, xT_d, wq_d, wk_d, wv_d, wo_d, cos_d, sin_d,
              perm_d, ones_d, msk_d, id_d, y_d)
    nc.compile()
    return nc


def _proj_segment(nc, tc, pa, wsb, lc, xT, wq_sb, wk_sb, wv_sb,
                  cos_sb, sin_sb, perm_sb, id_sb, qT, kT, vn,
                  deferred=None):
    """Q/K/V projections + RoPE for one 512-query chunk.

    Uses only PSUM banks 0-3 (prj x2, qrot, vtp) so the previous
    attention chunk's po/ps banks (4-7) stay untouched: its deferred
    normalization chain (``deferred``) is emitted after the first
    projection group and overlaps this segment's matmuls instead of
    stalling the pool-open."""
    ls = slice(lc * 512, (lc + 1) * 512)
    for ei, et in enumerate((4, 5, 0, 1, 2, 3)):   # k, v, then 4 q heads
        if et < 4:
            w_sl = lambda d_: wq_sb[:, et, d_, :]
        elif et == 4:
            w_sl = lambda d_: wk_sb[:, d_, :]
        else:
            w_sl = lambda d_: wv_sb[:, d_, :]

        if ei == 1 and deferred:
            for fn in deferred:
                fn()
            deferred = None
        prj = pa.tile([P, 512], _F32, tag="prj", bufs=2,
                      name=f"prj_{lc}_{et}")
        for dti in range(NDT):
            nc.tensor.matmul(prj[:], w_sl(dti), xT[:, dti, ls],
                             start=(dti == 0), stop=(dti == NDT - 1))
        qs = wsb.tile([P, 512], _BF, tag="qs", bufs=3, name=f"qs_{lc}_{et}")
        nc.vector.tensor_copy(qs[:], prj[:])
        if et == 5:
            vtp = pa.tile([P, 512], _BF, tag="vtp", bufs=1,
                          name=f"vtp_{lc}")
            for j in range(4):
                nc.tensor.matmul(vtp[:, j * P:(j + 1) * P],
                                 qs[:, j * P:(j + 1) * P], id_sb[:],
                                 is_transpose=True, skip_group_check=True)
            nc.vector.tensor_copy(vn[:, lc * 4:lc * 4 + 4, :],
                                  vtp[:].rearrange("p (a b) -> p a b", a=4))
        else:
            qrot = pa.tile([P, 512], _F32, tag="qrot", bufs=1,
                           name=f"qrot_{lc}_{et}")
            nc.tensor.matmul(qrot[:], perm_sb[:], qs[:], start=True,
                             stop=True)
            tt = wsb.tile([P, 512], _BF, tag="tt", bufs=2,
                          name=f"tt_{lc}_{et}")
            nc.vector.tensor_mul(tt[:], qs[:], cos_sb[:, ls])
            uu = wsb.tile([P, 512], _BF, tag="uu", bufs=2,
                          name=f"uu_{lc}_{et}")
            nc.vector.tensor_mul(uu[:], qrot[:], sin_sb[:, ls])
            dest = qT[:, et, ls] if et < 4 else kT[:, ls]
            nc.vector.tensor_add(dest, tt[:], uu[:])


def _op_group(nc, pool, wsb, lt, mc, oT, wo_sb, y_d, bufs, on_act=False):
    """One output-projection PSUM group: 4 head-matmuls -> copy -> DMA.

    The PSUM->SBUF copy runs on DVE by default; ``on_act`` routes it to
    the ACT engine for windows where DVE is serialized on the softmax
    normalization chain (pair drains, final phase)."""
    py = pool.tile([P, 512], _F32, tag="py", bufs=bufs,
                   name=f"py_{lt}_{mc}")
    for h in range(NHL):
        nc.tensor.matmul(py[:], oT[:, h, lt * P:(lt + 1) * P],
                         wo_sb[:, h, mc * 512:(mc + 1) * 512],
                         start=(h == 0), stop=(h == NHL - 1))
    ysb = wsb.tile([P, 512], _F32, tag="ysb", bufs=4, name=f"ysb_{lt}_{mc}")
    if on_act:
        nc.scalar.activation(ysb[:], py[:], _COPY)
    else:
        nc.vector.tensor_copy(ysb[:], py[:])
    nc.sync.dma_start(y_d[lt * P:(lt + 1) * P, mc * 512:(mc + 1) * 512],
                      ysb[:])


def _attn_chunk(nc, tc, pb, wsb, qi, qT, kT, vn, oT, ones_sb, msk_sb,
                id_sb, op_iter, op_args):
    """Causal attention for one 512-query chunk, all 4 heads.

    Two heads run in a software pipeline over key tiles so the PE never
    waits on the ACT exp of the tile it is about to consume.  When
    ``op_iter`` is set, one output-projection group of the previous
    chunk is interleaved per round as additional exp-latency cover
    (lookahead drops to 1 to fit PSUM: sc3+po2+ps2+py1 banks).
    """
    q0 = qi * 512
    nvis = 4 * qi
    nkt = nvis + 4
    look = 2 if op_iter is None else 1
    sc_bufs = 4 if op_iter is None else 2
    ops = list(op_iter) if op_iter is not None else []
    deferred = []
    # chunks 1-2 compute softmax denominators on the idle GPSIMD engine
    # (partition reduces of es) instead of PE ones-matmuls; chunk 0 is
    # small and chunk 3's reduce backlog would stall the final phase
    use_gp = qi in (1, 2)

    def op_left():
        return len(ops)

    def emit_op(on_act=True):
        if ops:
            lt, mc = ops.pop(0)
            _op_group(nc, pb, wsb, lt, mc, *op_args, bufs=2, on_act=on_act)

    for pair in ((0, 1), (2, 3)):
        # po/ps tiles are allocated lazily (at the first consumer round)
        # so the sc/py tags claim the low PSUM banks: the next segment's
        # pool then reuses early-freed banks first instead of WAR-waiting
        # on the pair-end reciprocal/normalization reads of po/ps.
        po = {}
        ps = {}
        es = {}
        if use_gp:
            for h in pair:
                ps[h] = wsb.tile([P, 512], _F32, tag="acc", bufs=4,
                                 name=f"acc_{qi}_{h}")

        def get_po_ps(h):
            if h not in po:
                po[h] = pb.tile([P, 512], _F32, tag="po",
                                bufs=(4 if use_gp else 2),
                                name=f"po_{qi}_{h}")
                if not use_gp:
                    ps[h] = pb.tile([P, 512], _F32, tag="ps", bufs=2,
                                    name=f"ps_{qi}_{h}")
            return po[h], ps[h]

        def emit_s(h, kt):
            off = max(0, (kt - nvis) * P)
            cs = slice(off, 512)
            sc = pb.tile([P, 512], _F32, tag="sc", bufs=sc_bufs,
                         name=f"sc_{qi}_{h}_{kt}")
            ktile = kT[:, kt * P:(kt + 1) * P]
            qtile = lambda o: qT[:, h, q0 + o:q0 + 512]
            if kt >= nvis:
                # diagonal tile: mask const first, S accumulates on top
                nc.tensor.matmul(sc[:, off:off + P], id_sb[:], msk_sb[:],
                                 start=True, stop=False,
                                 skip_group_check=True)
                nc.tensor.matmul(sc[:, off:off + P], ktile,
                                 qT[:, h, q0 + off:q0 + off + P],
                                 start=False, stop=True,
                                 skip_group_check=True)
                if off + P < 512:
                    nc.tensor.matmul(sc[:, off + P:512], ktile,
                                     qtile(off + P), start=True, stop=True,
                                     skip_group_check=True)
            else:
                nc.tensor.matmul(sc[:, cs], ktile, qtile(off),
                                 start=True, stop=True,
                                 skip_group_check=True)
            e = wsb.tile([P, 512], _BF, tag="es", bufs=8,
                         name=f"es_{qi}_{h}_{kt}")
            nc.scalar.activation(e[:, cs], sc[:, cs], _EXP, scale=SM_SCALE)
            if use_gp:
                # partition all-reduce of this key-tile's exp sums on the
                # otherwise idle GPSIMD engine, accumulated over key
                # tiles on DVE; replaces the PE ones-matmul
                gsum = wsb.tile([P, 512], _F32, tag="gsum", bufs=4,
                                name=f"gsum_{qi}_{h}_{kt}")
                nc.gpsimd.partition_all_reduce(
                    gsum[:, cs], e[:, cs], channels=P,
                    reduce_op=bass_isa.ReduceOp.add)
                if kt == 0:
                    nc.vector.tensor_copy(ps[h][:], gsum[:])
                else:
                    nc.vector.tensor_add(ps[h][:, cs], ps[h][:, cs],
                                         gsum[:, cs])
            es[(h, kt)] = e

        def emit_c(h, kt):
            off = max(0, (kt - nvis) * P)
            cs = slice(off, 512)
            e = es.pop((h, kt))
            poh, psh = get_po_ps(h)
            if not use_gp:
                nc.tensor.matmul(psh[:, cs], ones_sb[:], e[:, cs],
                                 start=(kt == 0), stop=(kt == nkt - 1),
                                 skip_group_check=True)
            nc.tensor.matmul(poh[:, cs], vn[:, kt, :], e[:, cs],
                             start=(kt == 0), stop=(kt == nkt - 1),
                             skip_group_check=True)

        reserve = 6 if pair[0] == 0 else 3
        for kt in range(nkt + look):
            if kt < nkt:
                for h in pair:
                    emit_s(h, kt)
            if op_left() > reserve:
                emit_op()
            if kt >= look:
                for h in pair:
                    emit_c(h, kt - look)

        def normalize(h, poh, psh):
            def fn():
                rec = wsb.tile([P, 512], _F32, tag="rec", bufs=2,
                               name=f"rec_{qi}_{h}")
                nc.vector.reciprocal(rec[:], psh[:])
                nc.vector.tensor_mul(oT[:, h, q0:q0 + 512], poh[:], rec[:])
            return fn

        if pair[0] == 0 or qi == NLC - 1:
            for h in pair:
                normalize(h, po[h], ps.get(h))()
        else:
            # pair 2's normalization chain is deferred into the next
            # segment (which only touches PSUM banks 0-3) so the pool
            # boundary doesn't stall on it
            for h in pair:
                deferred.append(normalize(h, po[h], ps.get(h)))
        # emit held-back OP groups under the pair-drain window so the
        # PE stays busy while DVE/ACT drain the pool's last reads
        while op_left() > (3 if pair[0] == 0 else 0):
            emit_op()

    return deferred


def _body(nc, tc, xT_d, wq_d, wk_d, wv_d, wo_d, cos_d, sin_d,
          perm_d, ones_d, msk_d, id_d, y_d):
    from contextlib import ExitStack
    ctx = ExitStack()
    with ctx:
        pp = ctx.enter_context(tc.tile_pool(name="persist", bufs=1))
        wsb = ctx.enter_context(tc.tile_pool(name="wsb", bufs=2))

        xT = pp.tile([P, NDT, L], _BF, tag="xT")
        wq_sb = pp.tile([P, NHL, NDT, 128], _BF, tag="wq")
        wk_sb = pp.tile([P, NDT, 128], _BF, tag="wk")
        wv_sb = pp.tile([P, NDT, 128], _BF, tag="wv")
        wo_sb = pp.tile([P, NHL, L], _BF, tag="wo")
        cos_sb = pp.tile([P, L], _BF, tag="cos")
        sin_sb = pp.tile([P, L], _BF, tag="sin")
        perm_sb = pp.tile([P, P], _BF, tag="perm")
        ones_sb = pp.tile([P, P], _BF, tag="ones")
        msk_sb = pp.tile([P, P], _BF, tag="msk")
        id_sb = pp.tile([P, P], _BF, tag="ident")
        qT = pp.tile([P, NHL, L], _BF, tag="qT")
        kT = pp.tile([P, L], _BF, tag="kT")
        vn = pp.tile([P, NKT, 128], _BF, tag="vn")
        oT = pp.tile([P, NHL, L], _BF, tag="oT")

        # DMA issue order tracks compute order (sync-engine DMAs are
        # FIFO and hold the SP sequencer while waiting on data).
        nc.sync.dma_start(wk_sb[:], wk_d[:])
        nc.sync.dma_start(xT[:, 0:4, 0:512], xT_d[:, 0:4, 0:512])
        nc.sync.dma_start(xT[:, 4:8, 0:512], xT_d[:, 4:8, 0:512])
        nc.sync.dma_start(xT[:, 8:16, 0:512], xT_d[:, 8:16, 0:512])
        nc.sync.dma_start(wv_sb[:], wv_d[:])
        nc.sync.dma_start(id_sb[:], id_d[:])
        nc.sync.dma_start(perm_sb[:], perm_d[:])
        nc.sync.dma_start(cos_sb[:], cos_d[:])
        nc.sync.dma_start(sin_sb[:], sin_d[:])
        for hq in range(4):
            nc.sync.dma_start(wq_sb[:, hq], wq_d[:, hq])
        nc.sync.dma_start(ones_sb[:], ones_d[:])
        nc.sync.dma_start(msk_sb[:], msk_d[:])
        for lc in range(1, NLC):
            ls = slice(lc * 512, (lc + 1) * 512)
            nc.sync.dma_start(xT[:, :, ls], xT_d[:, :, ls])
        nc.sync.dma_start(wo_sb[:], wo_d[:])

        # interleaved projection / attention chunks; attention chunk qi
        # also consumes the output-projection groups of chunk qi-1
        op_args = (oT, wo_sb, y_d)
        deferred = []
        for lc in range(NLC):
            with tc.tile_pool(name=f"pa{lc}", bufs=1, space="PSUM") as pa:
                _proj_segment(nc, tc, pa, wsb, lc, xT, wq_sb, wk_sb,
                              wv_sb, cos_sb, sin_sb, perm_sb, id_sb,
                              qT, kT, vn, deferred=deferred)
            op_iter = None
            if lc > 0:
                op_iter = iter([(lt, mc) for lt in range(4 * (lc - 1),
                                                        4 * lc)
                                for mc in range(4)])
            with tc.tile_pool(name=f"pb{lc}", bufs=1, space="PSUM") as pb:
                deferred = _attn_chunk(nc, tc, pb, wsb, lc, qT, kT, vn,
                                       oT, ones_sb, msk_sb, id_sb,
                                       op_iter, op_args)

        # last chunk's output projection, streamed out per 128x512 tile;
        # copies alternate ACT/DVE so the end-of-kernel copy backlog
        # drains on two engines
        with tc.tile_pool(name="pc", bufs=1, space="PSUM") as pc:
            n = 0
            for lt in range(12, 16):
                for mc in range(4):
                    _op_group(nc, pc, wsb, lt, mc, oT, wo_sb, y_d,
                              bufs=4, on_act=(n % 2 == 0))
                    n += 1


def host_constants():
    inv = (1.0 / (10000.0 ** (np.arange(0, HD, 2, dtype=np.float32) / HD))
           ).astype(np.float32)
    t = np.arange(L, dtype=np.float32)
    freqs = t[:, None] * inv[None, :]                    # [L, 64]
    emb = np.concatenate([freqs, freqs], axis=-1)        # [L, 128]
    cosT = np.ascontiguousarray(np.cos(emb).T).astype(BF16)
    sinT = np.ascontiguousarray(np.sin(emb).T).astype(BF16)
    perm = np.zeros((P, P), dtype=F32)
    for i in range(64):
        perm[i + 64, i] = -1.0      # qrot[d] = -q[d+64],  d < 64
        perm[i, i + 64] = 1.0       # qrot[d] =  q[d-64],  d >= 64
    ones = np.ones((P, P), dtype=F32)
    # msk[k, t] = MASK_NEG where key k > query t (strict upper part per
    # diagonal 128-block); added into PSUM before the S matmul.
    msk = np.where(np.arange(P)[:, None] > np.arange(P)[None, :],
                   MASK_NEG, 0.0).astype(F32)
    ident = np.eye(P, dtype=F32)
    return {
        "cosT": cosT, "sinT": sinT,
        "perm": perm.astype(BF16), "ones": ones.astype(BF16),
        "msk": msk.astype(BF16), "ident": ident.astype(BF16),
    }


def make_in_map(consts, x, Wq, Wk, Wv, Wo, b, g):
    qs = slice(g * 512, (g + 1) * 512)
    kvs = slice(g * 128, (g + 1) * 128)
    wq = np.ascontiguousarray(
        Wq[qs].T.reshape(NDT, P, NHL, 128).transpose(1, 2, 0, 3)
    ).astype(BF16)
    wk = np.ascontiguousarray(
        Wk[kvs].T.reshape(NDT, P, 128).transpose(1, 0, 2)).astype(BF16)
    wv = np.ascontiguousarray(
        Wv[kvs].T.reshape(NDT, P, 128).transpose(1, 0, 2)).astype(BF16)
    wo = np.ascontiguousarray(
        Wo[:, qs].T.reshape(NHL, P, D).transpose(1, 0, 2)).astype(BF16)
    xT = np.ascontiguousarray(
        x[b].T.reshape(NDT, P, L).transpose(1, 0, 2)).astype(BF16)
    return {
        "xT": xT,
        "wq": wq, "wk": wk, "wv": wv, "wo": wo,
        **consts,
    }


_NC_CACHE = {}


def get_nc():
    if "nc" not in _NC_CACHE:
        _NC_CACHE["nc"] = build_nc()
    return _NC_CACHE["nc"]


def kernel(x, Wq, Wk, Wv, Wo):
    x = np.asarray(x, dtype=F32)
    Wq = np.asarray(Wq, dtype=F32)
    Wk = np.asarray(Wk, dtype=F32)
    Wv = np.asarray(Wv, dtype=F32)
    Wo = np.asarray(Wo, dtype=F32)
    nc = get_nc()
    consts = host_constants()
    in_maps = [make_in_map(consts, x, Wq, Wk, Wv, Wo, c // 4, c % 4)
               for c in range(8)]
    res = run_bass_kernel_spmd(nc, in_maps, list(range(8)))
    outs = [r["y"].astype(np.float64) for r in res.results]
    y = np.stack([sum(outs[0:4]), sum(outs[4:8])], axis=0).astype(F32)
    return y
